# revision 28
# baseline (speedup 1.0000x reference)
"""Channel self-attention (b=8, n=4096, c=512, h=8, d=64) on 8 trn2 cores.

Sharding: data-parallel over batch — core i computes batch element i.
Host pre-transposes each shard to x[b].T ([512, 4096], contiguous) so the
contraction dim (c) lands on SBUF partitions with no on-chip transposes,
and pre-casts x / w_qkv / w_p to the matmul dtype (bf16 or f32r; PSUM
accumulation is fp32 either way — bf16 end-to-end rel err ~9e-3 vs the
fp32 reference, tolerance 2e-2).

Key algebraic trick (phase 2): attention is folded into the output
projection.  out = y @ w_p with y[n,(h,d)] = sum_e attn_h[d,e] v[n,(h,e)]
equals v @ wp' where wp'[(h,e),j] = sum_d attn_h[d,e] w_p[(h,d),j].
wp' costs 4 pair-packed matmuls (2048 PE cycles) and removes the whole
attn@v pass (16K cycles), the 8 PE transposes of attn, and 32 large
PSUM->SBUF copies.

Per-core dataflow:
  phase 1, per 512-token slab:
     xT tile [128, 4cc, 512]  (DMA)
     q,k  [128tok, 512] = sum_cc xT[cc].T @ w_{q,k}[cc]      (xT stationary)
     vT   [128ch, 512tok] = sum_cc w_v[cc, echunk].T @ xT[cc] (w_v stationary)
     scores (PSUM-accumulated over all 32 subtiles, per head-pair):
        bf16: s_pair [128(2h x 64d), 128(2h x 64e)] += q[:,pair].T @ k[:,pair]
        f32r: 256-wide k window (f32r needs >=256 moving rows for full rate)
  softmax: extract 8 [64,64] head blocks (x temperature), batched softmax
     over the free dim, copy into block-diagonal attn tiles (2 heads/tile).
  phase 2:
     wp'[pair] [128(2h x 64e), 512] = blockdiag(attn)[pair].T @ w_p[pair]
     out [128tok, 512] = sum_pair vT[pair,tok].T @ wp'[pair] + b_p, DMA out.
"""

import numpy as np

import concourse.bass as bass
import concourse.mybir as mybir
import concourse.tile as tile
from concourse.bass_utils import run_bass_kernel_spmd
from concourse.vector_clock import ScopedClock

f32 = mybir.dt.float32
f32r = mybir.dt.float32r
bf16 = mybir.dt.bfloat16
AX = mybir.AxisListType
AF = mybir.ActivationFunctionType

B, N, C = 8, 4096, 512
H, D = 8, 64
CC = C // 128          # 4 contraction chunks
SLAB = 512             # tokens per phase loop iteration
NSLAB = N // SLAB      # 8
NSUB = SLAB // 128     # 4


# ---------------------------------------------------------------------------
# Workaround: this walrus build allows 1 sync wait per instruction (2 on
# EventSemaphore), but TileContext's tail attaches every end-of-kernel wait to
# a single Drain.  Redistribute onto single-wait EventSemaphore instructions.
def _drain_and_barrier_split(self, tick_clock, wait_clock):
    nc = self.nc
    dummy = mybir.InstDrain(name=f"I-waitprobe-{nc.next_id()}", ins=[], outs=[])
    dummy.engine = mybir.EngineType.SP
    wait_clock.add_sem_waits(dummy, ScopedClock({None: tick_clock.global_clock}))
    num2handle = {h.num: h for h in self.sems.allocated().values()}
    for w in dummy.sync_info.on_wait:
        assert w.wait_mode == "sem-ge-imm", w
        nc.sync.wait_ge(num2handle[w.id], w.wait_value)
    nc.sync.drain()
    nc.all_engine_barrier()
    assert self.sems is not None
    popped = nc._tile_sem_poison_stack.pop()
    assert popped is self._sem_poison
    nc.clear_and_free_semaphores(list(self.sems.allocated().values()))
    nc.all_engine_barrier()


tile.TileContext._drain_and_barrier = _drain_and_barrier_split


# Same walrus limit, applied generally: Tile's add_semaphores can attach
# several waits to one instruction.  Split the excess onto EventSemaphore
# instructions (capacity 2) inserted just before, on the same engine, at BIR
# JSON serialization time so both the compile and bass2jax paths see it.
def _split_excess_waits_json(j):
    import copy

    for fn in j.get("functions", []):
        for bb in fn.get("blocks", []):
            new_insts = []
            for ins in bb.get("instructions", []):
                si = ins.get("sync_info") or {}
                waits = si.get("on_wait") or []
                cap = 2 if ins.get("opcode") == "EventSemaphore" else 1
                if len(waits) > cap:
                    keep = waits[-cap:]
                    excess = waits[:-cap]
                    for i in range(0, len(excess), 2):
                        new_insts.append(
                            {
                                "engine": ins["engine"],
                                "ins": [],
                                "outs": [],
                                "name": f"{ins['name']}-wsp{i}",
                                "opcode": "EventSemaphore",
                                "sync_info": {
                                    "on_update": [],
                                    "on_wait": excess[i : i + 2],
                                },
                            }
                        )
                    si = copy.deepcopy(si)
                    si["on_wait"] = keep
                    ins["sync_info"] = si
                new_insts.append(ins)
            bb["instructions"] = new_insts
    return j


_orig_to_json_bytes = bass.Bass.to_json_bytes


def _patched_to_json_bytes(self):
    import json as _json

    j = _json.loads(_orig_to_json_bytes(self))
    j = _split_excess_waits_json(j)
    return _json.dumps(j).encode()


bass.Bass.to_json_bytes = _patched_to_json_bytes


# walrus's LDWEIGHTS optimization (pinned off in bass_utils) helps the f32r
# variant (~30us of per-matmul weight-load overhead, self-loading matmuls)
# but hard-errors on the explicit InstLdweights pairs bf16 matmuls lower to.
# Flip it per-build via the module-global _LDW_OPT at compile time.
import concourse.bass_utils as _BU

_LDW_OPT = False
_orig_run_command = _BU.run_command


def _run_command_ldwopt(argv, **kw):
    if _LDW_OPT:
        argv = [
            "--enable-ldw-opt=true" if x == "--enable-ldw-opt=false" else x
            for x in argv
        ]
    return _orig_run_command(argv, **kw)


_BU.run_command = _run_command_ldwopt
# ---------------------------------------------------------------------------


def _bcast_ap(handle, offset, ap):
    base = handle[:]
    return bass.AP(tensor=base.tensor, offset=offset, ap=ap)


def _build(
    has_bqkv: bool, has_bp: bool, repeat: int = 1, dt=bf16
) -> bass.Bass:
    nc = bass.Bass()

    xt = nc.dram_tensor("xt", [C, N], dt, kind="ExternalInput")
    w_qkv = nc.dram_tensor("w_qkv", [C, 3 * C], dt, kind="ExternalInput")
    b_qkv = nc.dram_tensor("b_qkv", [3 * C], f32, kind="ExternalInput")
    w_p = nc.dram_tensor("w_p", [C, C], dt, kind="ExternalInput")
    b_p = nc.dram_tensor("b_p", [C], f32, kind="ExternalInput")
    temp = nc.dram_tensor("temperature", [H, 1, 1], f32, kind="ExternalInput")
    if repeat > 1:
        # structurally distinguishes the repeat-variant HLO so the neuron
        # compile cache cannot alias it to the repeat=1 NEFF
        salt = nc.dram_tensor("salt", [repeat], f32, kind="ExternalInput")
    out = nc.dram_tensor("out", [N, C], f32, kind="ExternalOutput")

    # scores: bf16 runs 1 row/cycle at any moving size => exact 128-wide
    # 2-head windows; f32r needs >=256 moving rows => 256-wide windows.
    SN = 128 if dt is not f32r else 256

    xt_r = xt[:].rearrange("(cc p) n -> p cc n", p=128)
    wqkv_r = w_qkv[:].rearrange("(cc p) j -> p cc j", p=128)
    wp_r = w_p[:].rearrange("(cc p) j -> p cc j", p=128)

    with tile.TileContext(nc) as tc:
        with (
            tc.tile_pool(name="consts", bufs=1) as consts,
            tc.tile_pool(name="vtp", bufs=1) as vtp,
            tc.tile_pool(name="attnp", bufs=1) as attnp,
        ):
            # ---- constants ----
            # weights go on SWDGE (gpsimd) so they stream in parallel with the
            # HWDGE x-tile loads; q/k chunks split per-cc so PE starts early.
            # separate tiles per matrix so q matmuls gate only on wq's DMAs
            wq_t = consts.tile([128, CC, C], dt)
            wk_t = consts.tile([128, CC, C], dt)
            wv_t = consts.tile([128, CC, C], dt)
            # slab 0 of x rides HWDGE ahead of/with the first weight chunks
            # (the SWDGE path pays ~1.3us of gpsimd descriptor-gen per DMA,
            # which would gate the very first Ldweights); emission order =
            # first-subtile consumption order: xt[t0], wq0, wk0, xt[t1], ...
            # HWDGE descriptor processing is ~630ns/DMA and serial, so the
            # head wants few, medium-size DMAs in consumption order: x slab 0
            # and wq/wk in halves (2 cc chunks each), then slab 1, wv, wp
            xt0 = consts.tile([128, CC, SLAB], dt)
            xt1 = consts.tile([128, CC, SLAB], dt)
            for hh in range(2):
                nc.sync.dma_start(
                    out=xt0[:, :, hh * 256 : (hh + 1) * 256],
                    in_=xt_r[:, :, hh * 256 : (hh + 1) * 256],
                )
                nc.sync.dma_start(
                    out=wq_t[:, 2 * hh : 2 * hh + 2, :],
                    in_=wqkv_r[:, 2 * hh : 2 * hh + 2, 0:C],
                )
                nc.sync.dma_start(
                    out=wk_t[:, 2 * hh : 2 * hh + 2, :],
                    in_=wqkv_r[:, 2 * hh : 2 * hh + 2, C : 2 * C],
                )
            nc.sync.dma_start(out=xt1, in_=xt_r[:, :, SLAB : 2 * SLAB])
            nc.sync.dma_start(out=wv_t, in_=wqkv_r[:, :, 2 * C : 3 * C])
            wp_t = consts.tile([128, CC, C], dt)
            nc.sync.dma_start(out=wp_t, in_=wp_r)
            temp_t = consts.tile([64, H], f32)
            nc.gpsimd.dma_start(out=temp_t, in_=_bcast_ap(temp, 0, [[0, 64], [1, H]]))
            if repeat > 1:
                salt_t = consts.tile([1, repeat], f32)
                nc.gpsimd.dma_start(out=salt_t, in_=salt[:][None, :])
            bd = consts.tile([128, 4, 128], dt)  # block-diag attn per pair
            zeros_t = consts.tile([128, 128], f32)
            nc.gpsimd.memset(zeros_t, 0.0)
            nc.vector.tensor_copy(
                out=bd,
                in_=bass.AP(
                    tensor=zeros_t.tensor,
                    offset=zeros_t.offset,
                    ap=[zeros_t.ap[0], [0, 4], zeros_t.ap[1]],
                ),
            )
            if has_bqkv:
                bqk_t = consts.tile([128, 2 * C], f32)
                nc.gpsimd.dma_start(
                    out=bqk_t, in_=_bcast_ap(b_qkv, 0, [[0, 128], [1, 2 * C]])
                )
                bv_t = consts.tile([128, CC], f32)
                nc.gpsimd.dma_start(
                    out=bv_t, in_=_bcast_ap(b_qkv, 2 * C, [[1, 128], [128, CC]])
                )
            if has_bp:
                bp_t = consts.tile([128, C], f32)
                nc.gpsimd.dma_start(
                    out=bp_t, in_=_bcast_ap(b_p, 0, [[0, 128], [1, C]])
                )

            for _rep in range(repeat):
                vt = vtp.tile([128, 4, N], dt)  # v^T: [pair-chunk rows, pair, token]

                with tc.tile_pool(name="spsum", bufs=1, space="PSUM") as spsum:
                    s_ps = [
                        spsum.tile([128, SN], f32, tag=f"s{p}", name=f"s{p}")
                        for p in range(4)
                    ]

                    # ================= phase 1 =================
                    with (
                        tc.tile_pool(name="xp", bufs=4) as xp,
                        tc.tile_pool(name="qkp", bufs=2) as qkp,
                        tc.tile_pool(name="qkps", bufs=1, space="PSUM") as qkps,
                        tc.tile_pool(name="vps", bufs=2, space="PSUM") as vps,
                    ):
                        NIT = NSLAB * NSUB

                        def emit_scores(q_sb, k_sb, it):
                            # stationary 2-head q block, moving k window
                            for p in range(4):
                                if SN == 128:
                                    kc0 = 128 * p
                                else:
                                    kc0 = 128 * p if p < 3 else 256
                                nc.tensor.matmul(
                                    s_ps[p],
                                    q_sb[:, p * 128 : (p + 1) * 128],
                                    k_sb[:, kc0 : kc0 + SN],
                                    start=(it == 0),
                                    stop=(it == NIT - 1),
                                )

                        def emit_v(s, xt_t):
                            # v^T chunks: w_v stationary, xT moving (N=512)
                            n0 = s * SLAB
                            for e in range(4):
                                v_ps = vps.tile([128, SLAB], f32, tag="v", name="v_ps")
                                for cc in range(CC):
                                    nc.tensor.matmul(
                                        v_ps,
                                        wv_t[:, cc, e * 128 : (e + 1) * 128],
                                        xt_t[:, cc, :],
                                        start=(cc == 0),
                                        stop=(cc == CC - 1),
                                    )
                                dst = vt[:, e, n0 : n0 + SLAB]
                                if has_bqkv:
                                    nc.vector.tensor_scalar_add(
                                        out=dst, in0=v_ps, scalar1=bv_t[:, e : e + 1]
                                    )
                                elif e % 2 == 0:
                                    nc.scalar.copy(out=dst, in_=v_ps)
                                else:
                                    nc.vector.tensor_copy(out=dst, in_=v_ps)

                        pending = None  # (q_sb, k_sb, it) one subtile behind
                        v_queue = []  # (s, xt_t), two slabs behind
                        for s in range(NSLAB):
                            n0 = s * SLAB
                            if _rep == 0 and s <= 1:
                                # already staged via the consts-pool tiles
                                xt_t = xt0 if s == 0 else xt1
                            else:
                                xt_t = xp.tile([128, CC, SLAB], dt, name="xt_t")
                                if s == 0:
                                    for t in range(NSUB):
                                        nc.gpsimd.dma_start(
                                            out=xt_t[:, :, t * 128 : (t + 1) * 128],
                                            in_=xt_r[:, :, n0 + t * 128 : n0 + (t + 1) * 128],
                                        )
                                else:
                                    # two halves: the next slab's first subtiles
                                    # gate on half the transfer
                                    hw_ = SLAB // 2
                                    for hh in range(2):
                                        nc.gpsimd.dma_start(
                                            out=xt_t[:, :, hh * hw_ : (hh + 1) * hw_],
                                            in_=xt_r[
                                                :, :, n0 + hh * hw_ : n0 + (hh + 1) * hw_
                                            ],
                                        )

                            # q, k per 128-token subtile; scores lag one subtile so
                            # PE never waits on the q/k PSUM->SBUF copies.
                            for t in range(NSUB):
                                it = s * NSUB + t
                                q_ps = qkps.tile([128, C], f32, tag="q", name="q_ps")
                                k_ps = qkps.tile([128, C], f32, tag="k", name="k_ps")
                                for cc in range(CC):
                                    lhs = xt_t[:, cc, t * 128 : (t + 1) * 128]
                                    nc.tensor.matmul(
                                        q_ps, lhs, wq_t[:, cc, :],
                                        start=(cc == 0), stop=(cc == CC - 1),
                                    )
                                    nc.tensor.matmul(
                                        k_ps, lhs, wk_t[:, cc, :],
                                        start=(cc == 0), stop=(cc == CC - 1),
                                    )
                                q_sb = qkp.tile([128, C], dt, tag="q_sb", name="q_sb")
                                k_sb = qkp.tile([128, C], dt, tag="k_sb", name="k_sb")
                                if has_bqkv:
                                    nc.vector.tensor_add(out=q_sb, in0=q_ps, in1=bqk_t[:, 0:C])
                                    nc.vector.tensor_add(out=k_sb, in0=k_ps, in1=bqk_t[:, C : 2 * C])
                                else:
                                    nc.scalar.copy(out=q_sb, in_=q_ps)
                                    nc.vector.tensor_copy(out=k_sb, in_=k_ps)
                                if pending is not None:
                                    emit_scores(*pending)
                                pending = (q_sb, k_sb, it)

                            v_queue.append((s, xt_t))
                            if len(v_queue) > 2:
                                emit_v(*v_queue.pop(0))
                        emit_scores(*pending)
                        for args in v_queue:
                            emit_v(*args)

                    # ============ softmax (fused from PSUM) ============
                    # writes normalized bf16/f32r attn straight into the
                    # block-diagonal tiles bd[pair] (off-diag stays zero)
                    attn = attnp.tile([64, H, 64], f32)
                    m = attnp.tile([64, H], f32)
                    ssum = attnp.tile([64, H], f32)
                    for h in range(H):
                        p = h // 2
                        r0 = (h % 2) * 64
                        c0 = (h % 2) * 64 + (128 if (SN == 256 and p == 3) else 0)
                        blk = s_ps[p][r0 : r0 + 64, c0 : c0 + 64]
                        nc.vector.reduce_max(out=m[:, h : h + 1], in_=blk, axis=AX.X)
                        # m <- -(temp * max)
                        nc.vector.tensor_scalar(
                            out=m[:, h : h + 1], in0=m[:, h : h + 1],
                            scalar1=temp_t[:, h : h + 1], scalar2=-1.0,
                            op0=mybir.AluOpType.mult, op1=mybir.AluOpType.mult,
                        )
                        # attn_h = exp(temp*s - temp*max), row sums into ssum
                        nc.scalar.activation(
                            out=attn[:, h, :], in_=blk, func=AF.Exp,
                            bias=m[:, h : h + 1], scale=temp_t[:, h : h + 1],
                            accum_out=ssum[:, h : h + 1],
                        )
                    nc.vector.reciprocal(out=ssum, in_=ssum)
                    attn_b = attnp.tile([64, H, 64], dt)
                    for h in range(H):
                        # normalize then immediately stage into the block-diag
                        # tile, Act/DVE alternating, so bd pairs complete
                        # progressively and wp' matmuls can start early
                        nc.vector.tensor_scalar_mul(
                            out=attn_b[:, h, :],
                            in0=attn[:, h, :],
                            scalar1=ssum[:, h : h + 1],
                        )
                        o = (h % 2) * 64
                        if h % 2 == 0:
                            nc.scalar.copy(
                                out=bd[o : o + 64, h // 2, o : o + 64],
                                in_=attn_b[:, h, :],
                            )
                        else:
                            nc.vector.tensor_copy(
                                out=bd[o : o + 64, h // 2, o : o + 64],
                                in_=attn_b[:, h, :],
                            )

                # ================= phase 2 =================
                # wp'[pair] = blockdiag(attn)[pair].T @ w_p[pair], then
                # out[tok,:] = sum_pair vT[pair,tok].T @ wp'[pair]
                with (
                    tc.tile_pool(name="wpsb", bufs=1) as wpsb,
                    tc.tile_pool(name="wps", bufs=2, space="PSUM") as wps,
                    # osb bufs must stay even: copies alternate Act/DVE per
                    # subtile, so even reuse distance keeps WAW deps within
                    # one (in-order) engine instead of cross-engine ping-pong
                    tc.tile_pool(name="osb", bufs=4) as osb,
                    tc.tile_pool(name="osp", bufs=6, space="PSUM") as osp,
                ):
                    wpp = wpsb.tile([128, 4, C], dt)
                    for p in range(4):
                        w_ps = wps.tile([128, C], f32, tag="wpp", name="w_ps")
                        nc.tensor.matmul(
                            w_ps, bd[:, p, :], wp_t[:, p, :], start=True, stop=True
                        )
                        if p % 2 == 0:
                            nc.scalar.copy(out=wpp[:, p, :], in_=w_ps)
                        else:
                            nc.vector.tensor_copy(out=wpp[:, p, :], in_=w_ps)

                    def emit_proj(s):
                        n0 = s * SLAB
                        for t in range(NSUB):
                            o_ps = osp.tile([128, C], f32, tag="o", name="o_ps")
                            for p in range(4):
                                nc.tensor.matmul(
                                    o_ps,
                                    vt[:, p, n0 + t * 128 : n0 + (t + 1) * 128],
                                    wpp[:, p, :],
                                    start=(p == 0), stop=(p == 3),
                                )
                            o_sb = osb.tile([128, C], f32, tag="o_sb", name="o_sb")
                            last = s == NSLAB - 1
                            if has_bp:
                                nc.vector.tensor_add(out=o_sb, in0=o_ps, in1=bp_t)
                                nc.sync.dma_start(
                                    out=out[:][n0 + t * 128 : n0 + (t + 1) * 128, :],
                                    in_=o_sb,
                                )
                            elif last:
                                # drain the tail in halves: DMA starts after half
                                # the copy, on alternating engines
                                for hh in range(2):
                                    csl = slice(hh * 256, (hh + 1) * 256)
                                    if (t + hh) % 2 == 0:
                                        nc.scalar.copy(out=o_sb[:, csl], in_=o_ps[:, csl])
                                    else:
                                        nc.vector.tensor_copy(
                                            out=o_sb[:, csl], in_=o_ps[:, csl]
                                        )
                                    nc.sync.dma_start(
                                        out=out[:][
                                            n0 + t * 128 : n0 + (t + 1) * 128, csl
                                        ],
                                        in_=o_sb[:, csl],
                                    )
                            else:
                                if t % 2 == 0:
                                    nc.scalar.copy(out=o_sb, in_=o_ps)
                                else:
                                    nc.vector.tensor_copy(out=o_sb, in_=o_ps)
                                nc.sync.dma_start(
                                    out=out[:][n0 + t * 128 : n0 + (t + 1) * 128, :],
                                    in_=o_sb,
                                )

                    for s in range(NSLAB):
                        emit_proj(s)

    return nc


_cache: dict = {}
last_results = None


def kernel(x, w_qkv, b_qkv, w_p, b_p, temperature):
    global last_results
    import os

    import ml_dtypes

    dt_name = os.environ.get("KSA_DT", "bf16")
    dt = {"bf16": bf16, "f32r": f32r}[dt_name]
    np_dt = ml_dtypes.bfloat16 if dt is bf16 else np.float32

    x = np.ascontiguousarray(np.asarray(x, dtype=np.float32))
    w_qkv = np.ascontiguousarray(np.asarray(w_qkv, dtype=np.float32).astype(np_dt))
    b_qkv = np.ascontiguousarray(np.asarray(b_qkv, dtype=np.float32))
    w_p = np.ascontiguousarray(np.asarray(w_p, dtype=np.float32).astype(np_dt))
    b_p = np.ascontiguousarray(np.asarray(b_p, dtype=np.float32))
    temperature = np.ascontiguousarray(np.asarray(temperature, dtype=np.float32))

    global _LDW_OPT
    _LDW_OPT = dt is f32r

    key = (bool(np.any(b_qkv)), bool(np.any(b_p)), dt_name)
    if key not in _cache:
        _cache[key] = _build(key[0], key[1], dt=dt)
    nc = _cache[key]

    in_maps = []
    for i in range(B):
        in_maps.append(
            {
                "xt": np.ascontiguousarray(x[i].T.astype(np_dt)),
                "w_qkv": w_qkv,
                "b_qkv": b_qkv,
                "w_p": w_p,
                "b_p": b_p,
                "temperature": temperature,
            }
        )

    trace = bool(int(os.environ.get("KSA_TRACE", "0")))
    res = run_bass_kernel_spmd(nc, in_maps, core_ids=list(range(B)), trace=trace)
    last_results = res
    return np.stack([res.results[i]["out"] for i in range(B)]).astype(np.float32)


# revision 45
# speedup vs baseline: 1.1091x; 1.1091x over previous
"""Channel self-attention (b=8, n=4096, c=512, h=8, d=64) on 8 trn2 cores.

Sharding: data-parallel over batch — core i computes batch element i.
Host pre-transposes each shard to x[b].T ([512, 4096], contiguous) so the
contraction dim (c) lands on SBUF partitions with no on-chip transposes,
and pre-casts x / w_qkv / w_p to the matmul dtype (bf16 or f32r; PSUM
accumulation is fp32 either way — bf16 end-to-end rel err ~9e-3 vs the
fp32 reference, tolerance 2e-2).

Key algebraic trick (phase 2): attention is folded into the output
projection.  out = y @ w_p with y[n,(h,d)] = sum_e attn_h[d,e] v[n,(h,e)]
equals v @ wp' where wp'[(h,e),j] = sum_d attn_h[d,e] w_p[(h,d),j].
wp' costs 4 pair-packed matmuls (2048 PE cycles) and removes the whole
attn@v pass (16K cycles), the 8 PE transposes of attn, and 32 large
PSUM->SBUF copies.

Per-core dataflow:
  phase 1, per 512-token slab:
     xT tile [128, 4cc, 512]  (DMA)
     q,k  [128tok, 512] = sum_cc xT[cc].T @ w_{q,k}[cc]      (xT stationary)
     vT   [128ch, 512tok] = sum_cc w_v[cc, echunk].T @ xT[cc] (w_v stationary)
     scores (PSUM-accumulated over all 32 subtiles, per head-pair):
        bf16: s_pair [128(2h x 64d), 128(2h x 64e)] += q[:,pair].T @ k[:,pair]
        f32r: 256-wide k window (f32r needs >=256 moving rows for full rate)
  softmax: extract 8 [64,64] head blocks (x temperature), batched softmax
     over the free dim, copy into block-diagonal attn tiles (2 heads/tile).
  phase 2:
     wp'[pair] [128(2h x 64e), 512] = blockdiag(attn)[pair].T @ w_p[pair]
     out [128tok, 512] = sum_pair vT[pair,tok].T @ wp'[pair] + b_p, DMA out.
"""

import numpy as np

import concourse.bass as bass
import concourse.mybir as mybir
import concourse.tile as tile
from concourse.bass_utils import run_bass_kernel_spmd
from concourse.vector_clock import ScopedClock

f32 = mybir.dt.float32
f32r = mybir.dt.float32r
bf16 = mybir.dt.bfloat16
AX = mybir.AxisListType
AF = mybir.ActivationFunctionType

B, N, C = 8, 4096, 512
H, D = 8, 64
CC = C // 128          # 4 contraction chunks
SLAB = 512             # tokens per phase loop iteration
NSLAB = N // SLAB      # 8
NSUB = SLAB // 128     # 4


# ---------------------------------------------------------------------------
# Workaround: this walrus build allows 1 sync wait per instruction (2 on
# EventSemaphore), but TileContext's tail attaches every end-of-kernel wait to
# a single Drain.  Redistribute onto single-wait EventSemaphore instructions.
def _drain_and_barrier_split(self, tick_clock, wait_clock):
    nc = self.nc
    dummy = mybir.InstDrain(name=f"I-waitprobe-{nc.next_id()}", ins=[], outs=[])
    dummy.engine = mybir.EngineType.SP
    wait_clock.add_sem_waits(dummy, ScopedClock({None: tick_clock.global_clock}))
    num2handle = {h.num: h for h in self.sems.allocated().values()}
    for w in dummy.sync_info.on_wait:
        assert w.wait_mode == "sem-ge-imm", w
        nc.sync.wait_ge(num2handle[w.id], w.wait_value)
    nc.sync.drain()
    nc.all_engine_barrier()
    assert self.sems is not None
    popped = nc._tile_sem_poison_stack.pop()
    assert popped is self._sem_poison
    nc.clear_and_free_semaphores(list(self.sems.allocated().values()))
    nc.all_engine_barrier()


tile.TileContext._drain_and_barrier = _drain_and_barrier_split


# Same walrus limit, applied generally: Tile's add_semaphores can attach
# several waits to one instruction.  Split the excess onto EventSemaphore
# instructions (capacity 2) inserted just before, on the same engine, at BIR
# JSON serialization time so both the compile and bass2jax paths see it.
def _split_excess_waits_json(j):
    import copy

    for fn in j.get("functions", []):
        for bb in fn.get("blocks", []):
            new_insts = []
            for ins in bb.get("instructions", []):
                si = ins.get("sync_info") or {}
                waits = si.get("on_wait") or []
                cap = 2 if ins.get("opcode") == "EventSemaphore" else 1
                if len(waits) > cap:
                    keep = waits[-cap:]
                    excess = waits[:-cap]
                    for i in range(0, len(excess), 2):
                        new_insts.append(
                            {
                                "engine": ins["engine"],
                                "ins": [],
                                "outs": [],
                                "name": f"{ins['name']}-wsp{i}",
                                "opcode": "EventSemaphore",
                                "sync_info": {
                                    "on_update": [],
                                    "on_wait": excess[i : i + 2],
                                },
                            }
                        )
                    si = copy.deepcopy(si)
                    si["on_wait"] = keep
                    ins["sync_info"] = si
                new_insts.append(ins)
            bb["instructions"] = new_insts
    return j


# Drop redundant PE weight loads: tile-legalize emits one InstLdweights per
# bf16 matmul, but adjacent q/k matmuls share the same stationary xt chunk —
# the second load re-streams identical weights (~53ns each on HW).  Safe to
# drop when the access pattern matches the previous Ldweights exactly, the
# instruction carries no sync, and only Matmults sit in between.
def _drop_redundant_ldweights_json(j):
    import json as _json

    for fn in j.get("functions", []):
        for bb in fn.get("blocks", []):
            insts = bb.get("instructions", [])
            keep = []
            prev_ldw_key = None
            for ins in insts:
                op = ins.get("opcode")
                eng = ins.get("engine")
                if eng == "PE" and op == "Ldweights":
                    si = ins.get("sync_info") or {}
                    key = _json.dumps(
                        [ins.get("ins"), ins.get("perf_mode"),
                         ins.get("is_transpose"), ins.get("tile_position")],
                        sort_keys=True,
                    )
                    if (
                        key == prev_ldw_key
                        and not si.get("on_wait")
                        and not si.get("on_update")
                    ):
                        continue  # redundant reload of identical weights
                    prev_ldw_key = key
                elif eng == "PE" and op == "Matmult":
                    pass  # matmuls don't disturb the loaded weights
                elif eng == "PE":
                    prev_ldw_key = None  # anything else on PE: be safe
                keep.append(ins)
            bb["instructions"] = keep
    return j


_orig_to_json_bytes = bass.Bass.to_json_bytes


def _patched_to_json_bytes(self):
    import json as _json
    import os as _os

    j = _json.loads(_orig_to_json_bytes(self))
    j = _split_excess_waits_json(j)
    if _os.environ.get("KSA_DEDUP_LDW", "1") == "1":
        j = _drop_redundant_ldweights_json(j)
    return _json.dumps(j).encode()


bass.Bass.to_json_bytes = _patched_to_json_bytes


# walrus's LDWEIGHTS optimization (pinned off in bass_utils) helps the f32r
# variant (~30us of per-matmul weight-load overhead, self-loading matmuls)
# but hard-errors on the explicit InstLdweights pairs bf16 matmuls lower to.
# Flip it per-build via the module-global _LDW_OPT at compile time.
import concourse.bass_utils as _BU

_LDW_OPT = False
_orig_run_command = _BU.run_command


def _run_command_ldwopt(argv, **kw):
    if _LDW_OPT:
        argv = [
            "--enable-ldw-opt=true" if x == "--enable-ldw-opt=false" else x
            for x in argv
        ]
    return _orig_run_command(argv, **kw)


_BU.run_command = _run_command_ldwopt
# ---------------------------------------------------------------------------


def _bcast_ap(handle, offset, ap):
    base = handle[:]
    return bass.AP(tensor=base.tensor, offset=offset, ap=ap)


def _build(
    has_bqkv: bool, has_bp: bool, repeat: int = 1, dt=bf16
) -> bass.Bass:
    nc = bass.Bass()

    xt = nc.dram_tensor("xt", [C, N], dt, kind="ExternalInput")
    w_qkv = nc.dram_tensor("w_qkv", [C, 3 * C], dt, kind="ExternalInput")
    b_qkv = nc.dram_tensor("b_qkv", [3 * C], f32, kind="ExternalInput")
    w_p = nc.dram_tensor("w_p", [C, C], dt, kind="ExternalInput")
    b_p = nc.dram_tensor("b_p", [C], f32, kind="ExternalInput")
    temp = nc.dram_tensor("temperature", [H, 1, 1], f32, kind="ExternalInput")
    if repeat > 1:
        # structurally distinguishes the repeat-variant HLO so the neuron
        # compile cache cannot alias it to the repeat=1 NEFF
        salt = nc.dram_tensor("salt", [repeat], f32, kind="ExternalInput")
    out = nc.dram_tensor("out", [N, C], f32, kind="ExternalOutput")

    # scores: bf16 runs 1 row/cycle at any moving size => exact 128-wide
    # 2-head windows; f32r needs >=256 moving rows => 256-wide windows.
    SN = 128 if dt is not f32r else 256

    xt_r = xt[:].rearrange("(cc p) n -> p cc n", p=128)
    wqkv_r = w_qkv[:].rearrange("(cc p) j -> p cc j", p=128)
    wp_r = w_p[:].rearrange("(cc p) j -> p cc j", p=128)

    with tile.TileContext(nc) as tc:
        with (
            tc.tile_pool(name="consts", bufs=1) as consts,
            tc.tile_pool(name="vtp", bufs=1) as vtp,
            tc.tile_pool(name="attnp", bufs=1) as attnp,
        ):
            # ---- constants ----
            # weights go on SWDGE (gpsimd) so they stream in parallel with the
            # HWDGE x-tile loads; q/k chunks split per-cc so PE starts early.
            # separate tiles per matrix so q matmuls gate only on wq's DMAs
            wq_t = consts.tile([128, CC, C], dt)
            wk_t = consts.tile([128, CC, C], dt)
            wv_t = consts.tile([128, CC, C], dt)
            # slab 0 of x rides HWDGE ahead of/with the first weight chunks
            # (the SWDGE path pays ~1.3us of gpsimd descriptor-gen per DMA,
            # which would gate the very first Ldweights); emission order =
            # first-subtile consumption order: xt[t0], wq0, wk0, xt[t1], ...
            # HWDGE descriptor processing is ~630ns/DMA and serial, so the
            # head wants few, medium-size DMAs in consumption order: x slab 0
            # and wq/wk in halves (2 cc chunks each), then slab 1, wv, wp
            xt0 = consts.tile([128, CC, SLAB], dt)
            xt1 = consts.tile([128, CC, SLAB], dt)
            if repeat > 1:
                # rep-parity ping-pong staging for slabs 0-1: consts-pool
                # tiles never collide with the per-rep SBUF stack churn, so
                # the next rep's x prefetch isn't WAR-blocked behind the
                # previous rep's phase-2 drain
                xt0b = consts.tile([128, CC, SLAB], dt)
                xt1b = consts.tile([128, CC, SLAB], dt)
            for hh in range(2):
                nc.sync.dma_start(
                    out=xt0[:, :, hh * 256 : (hh + 1) * 256],
                    in_=xt_r[:, :, hh * 256 : (hh + 1) * 256],
                )
                nc.sync.dma_start(
                    out=wq_t[:, 2 * hh : 2 * hh + 2, :],
                    in_=wqkv_r[:, 2 * hh : 2 * hh + 2, 0:C],
                )
                nc.sync.dma_start(
                    out=wk_t[:, 2 * hh : 2 * hh + 2, :],
                    in_=wqkv_r[:, 2 * hh : 2 * hh + 2, C : 2 * C],
                )
            nc.sync.dma_start(out=xt1, in_=xt_r[:, :, SLAB : 2 * SLAB])
            nc.sync.dma_start(out=wv_t, in_=wqkv_r[:, :, 2 * C : 3 * C])
            wp_t = consts.tile([128, CC, C], dt)
            nc.sync.dma_start(out=wp_t, in_=wp_r)
            temp_t = consts.tile([64, H], f32)
            nc.gpsimd.dma_start(out=temp_t, in_=_bcast_ap(temp, 0, [[0, 64], [1, H]]))
            if repeat > 1:
                salt_t = consts.tile([1, repeat], f32)
                nc.gpsimd.dma_start(out=salt_t, in_=salt[:][None, :])
            bd = consts.tile([128, 4, 128], dt)  # block-diag attn per pair
            zeros_t = consts.tile([128, 128], f32)
            nc.gpsimd.memset(zeros_t, 0.0)
            nc.vector.tensor_copy(
                out=bd,
                in_=bass.AP(
                    tensor=zeros_t.tensor,
                    offset=zeros_t.offset,
                    ap=[zeros_t.ap[0], [0, 4], zeros_t.ap[1]],
                ),
            )
            if has_bqkv:
                bqk_t = consts.tile([128, 2 * C], f32)
                nc.gpsimd.dma_start(
                    out=bqk_t, in_=_bcast_ap(b_qkv, 0, [[0, 128], [1, 2 * C]])
                )
                bv_t = consts.tile([128, CC], f32)
                nc.gpsimd.dma_start(
                    out=bv_t, in_=_bcast_ap(b_qkv, 2 * C, [[1, 128], [128, CC]])
                )
            if has_bp:
                bp_t = consts.tile([128, C], f32)
                nc.gpsimd.dma_start(
                    out=bp_t, in_=_bcast_ap(b_p, 0, [[0, 128], [1, C]])
                )

            for _rep in range(repeat):
                if _rep == 0:
                    xt_s0, xt_s1 = xt0, xt1
                else:
                    xt_s0, xt_s1 = (xt0, xt1) if _rep % 2 == 0 else (xt0b, xt1b)
                    # prefetch this rep's slabs 0-1 on SWDGE; the Pool queue
                    # reaches these right after the previous rep's slab DMAs,
                    # a full phase ahead of first use
                    for xt_sN, s in ((xt_s0, 0), (xt_s1, 1)):
                        for hh in range(2):
                            nc.gpsimd.dma_start(
                                out=xt_sN[:, :, hh * 256 : (hh + 1) * 256],
                                in_=xt_r[
                                    :, :, s * SLAB + hh * 256 : s * SLAB + (hh + 1) * 256
                                ],
                            )
                vt = vtp.tile([128, 4, N], dt)  # v^T: [pair-chunk rows, pair, token]

                with tc.tile_pool(name="spsum", bufs=1, space="PSUM") as spsum:
                    s_ps = [
                        spsum.tile([128, SN], f32, tag=f"s{p}", name=f"s{p}")
                        for p in range(4)
                    ]

                    # ================= phase 1 =================
                    with (
                        tc.tile_pool(name="xp", bufs=4) as xp,
                        tc.tile_pool(name="qkp", bufs=2) as qkp,
                        tc.tile_pool(name="qkps", bufs=1, space="PSUM") as qkps,
                        tc.tile_pool(name="vps", bufs=2, space="PSUM") as vps,
                    ):
                        NIT = NSLAB * NSUB

                        def emit_scores(q_sb, k_sb, it):
                            # stationary 2-head q block, moving k window
                            for p in range(4):
                                if SN == 128:
                                    kc0 = 128 * p
                                else:
                                    kc0 = 128 * p if p < 3 else 256
                                nc.tensor.matmul(
                                    s_ps[p],
                                    q_sb[:, p * 128 : (p + 1) * 128],
                                    k_sb[:, kc0 : kc0 + SN],
                                    start=(it == 0),
                                    stop=(it == NIT - 1),
                                )

                        def emit_v(s, xt_t, tail=False):
                            # v^T chunks: w_v stationary, xT moving (N=512)
                            n0 = s * SLAB
                            for e in range(4):
                                v_ps = vps.tile([128, SLAB], f32, tag="v", name="v_ps")
                                for cc in range(CC):
                                    nc.tensor.matmul(
                                        v_ps,
                                        wv_t[:, cc, e * 128 : (e + 1) * 128],
                                        xt_t[:, cc, :],
                                        start=(cc == 0),
                                        stop=(cc == CC - 1),
                                    )
                                dst = vt[:, e, n0 : n0 + SLAB]
                                if has_bqkv:
                                    nc.vector.tensor_scalar_add(
                                        out=dst, in0=v_ps, scalar1=bv_t[:, e : e + 1]
                                    )
                                elif tail:
                                    # the trailing slabs' copies overlap the
                                    # softmax on Act/DVE — run them on the
                                    # otherwise-idle Pool engine instead
                                    nc.gpsimd.tensor_copy(out=dst, in_=v_ps)
                                elif e % 2 == 0:
                                    nc.scalar.copy(out=dst, in_=v_ps)
                                else:
                                    nc.vector.tensor_copy(out=dst, in_=v_ps)

                        pending = None  # (q_sb, k_sb, it) one subtile behind
                        v_queue = []  # (s, xt_t), two slabs behind
                        for s in range(NSLAB):
                            n0 = s * SLAB
                            if s <= 1:
                                # staged via the consts-pool (parity) tiles
                                xt_t = xt_s0 if s == 0 else xt_s1
                            else:
                                xt_t = xp.tile([128, CC, SLAB], dt, name="xt_t")
                                # two halves: the next slab's first subtiles
                                # gate on half the transfer
                                hw_ = SLAB // 2
                                for hh in range(2):
                                    nc.gpsimd.dma_start(
                                        out=xt_t[:, :, hh * hw_ : (hh + 1) * hw_],
                                        in_=xt_r[
                                            :, :, n0 + hh * hw_ : n0 + (hh + 1) * hw_
                                        ],
                                    )

                            # q, k per 128-token subtile; scores lag one subtile so
                            # PE never waits on the q/k PSUM->SBUF copies.
                            for t in range(NSUB):
                                it = s * NSUB + t
                                q_ps = qkps.tile([128, C], f32, tag="q", name="q_ps")
                                k_ps = qkps.tile([128, C], f32, tag="k", name="k_ps")
                                for cc in range(CC):
                                    lhs = xt_t[:, cc, t * 128 : (t + 1) * 128]
                                    nc.tensor.matmul(
                                        q_ps, lhs, wq_t[:, cc, :],
                                        start=(cc == 0), stop=(cc == CC - 1),
                                    )
                                    nc.tensor.matmul(
                                        k_ps, lhs, wk_t[:, cc, :],
                                        start=(cc == 0), stop=(cc == CC - 1),
                                    )
                                q_sb = qkp.tile([128, C], dt, tag="q_sb", name="q_sb")
                                k_sb = qkp.tile([128, C], dt, tag="k_sb", name="k_sb")
                                if has_bqkv:
                                    nc.vector.tensor_add(out=q_sb, in0=q_ps, in1=bqk_t[:, 0:C])
                                    nc.vector.tensor_add(out=k_sb, in0=k_ps, in1=bqk_t[:, C : 2 * C])
                                else:
                                    nc.scalar.copy(out=q_sb, in_=q_ps)
                                    nc.vector.tensor_copy(out=k_sb, in_=k_ps)
                                if pending is not None:
                                    emit_scores(*pending)
                                pending = (q_sb, k_sb, it)

                            v_queue.append((s, xt_t))
                            if len(v_queue) > 2:
                                emit_v(*v_queue.pop(0))
                        emit_scores(*pending)
                        for args in v_queue:
                            emit_v(*args, tail=True)

                    # ============ softmax (fused from PSUM) ============
                    # writes normalized bf16/f32r attn straight into the
                    # block-diagonal tiles bd[pair] (off-diag stays zero)
                    attn = attnp.tile([64, H, 64], f32)
                    m = attnp.tile([64, H], f32)
                    ssum = attnp.tile([64, H], f32)
                    for h in range(H):
                        p = h // 2
                        r0 = (h % 2) * 64
                        c0 = (h % 2) * 64 + (128 if (SN == 256 and p == 3) else 0)
                        blk = s_ps[p][r0 : r0 + 64, c0 : c0 + 64]
                        nc.vector.reduce_max(out=m[:, h : h + 1], in_=blk, axis=AX.X)
                        # m <- -(temp * max)
                        nc.vector.tensor_scalar(
                            out=m[:, h : h + 1], in0=m[:, h : h + 1],
                            scalar1=temp_t[:, h : h + 1], scalar2=-1.0,
                            op0=mybir.AluOpType.mult, op1=mybir.AluOpType.mult,
                        )
                        # attn_h = exp(temp*s - temp*max), row sums into ssum
                        nc.scalar.activation(
                            out=attn[:, h, :], in_=blk, func=AF.Exp,
                            bias=m[:, h : h + 1], scale=temp_t[:, h : h + 1],
                            accum_out=ssum[:, h : h + 1],
                        )
                    nc.vector.reciprocal(out=ssum, in_=ssum)
                    attn_b = attnp.tile([64, H, 64], dt)
                    for h in range(H):
                        # normalize then immediately stage into the block-diag
                        # tile, Act/DVE alternating, so bd pairs complete
                        # progressively and wp' matmuls can start early
                        nc.vector.tensor_scalar_mul(
                            out=attn_b[:, h, :],
                            in0=attn[:, h, :],
                            scalar1=ssum[:, h : h + 1],
                        )
                        o = (h % 2) * 64
                        if h % 2 == 0:
                            nc.scalar.copy(
                                out=bd[o : o + 64, h // 2, o : o + 64],
                                in_=attn_b[:, h, :],
                            )
                        else:
                            nc.vector.tensor_copy(
                                out=bd[o : o + 64, h // 2, o : o + 64],
                                in_=attn_b[:, h, :],
                            )

                # ================= phase 2 =================
                # wp'[pair] = blockdiag(attn)[pair].T @ w_p[pair], then
                # out[tok,:] = sum_pair vT[pair,tok].T @ wp'[pair]
                with (
                    tc.tile_pool(name="wpsb", bufs=1) as wpsb,
                    # osp opens before wps so it sits at the PSUM stack base:
                    # the next rep's first-touched pools (spsum/qkps) then
                    # collide with osp banks whose last chains retire several
                    # subtiles before the rep ends, not with the very tail
                    tc.tile_pool(name="osp", bufs=6, space="PSUM") as osp,
                    tc.tile_pool(name="wps", bufs=2, space="PSUM") as wps,
                    # osb bufs must stay even: copies alternate Act/DVE per
                    # subtile, so even reuse distance keeps WAW deps within
                    # one (in-order) engine instead of cross-engine ping-pong
                    tc.tile_pool(name="osb", bufs=4) as osb,
                ):
                    wpp = wpsb.tile([128, 4, C], dt)
                    for p in range(4):
                        w_ps = wps.tile([128, C], f32, tag="wpp", name="w_ps")
                        nc.tensor.matmul(
                            w_ps, bd[:, p, :], wp_t[:, p, :], start=True, stop=True
                        )
                        if p % 2 == 0:
                            nc.scalar.copy(out=wpp[:, p, :], in_=w_ps)
                        else:
                            nc.vector.tensor_copy(out=wpp[:, p, :], in_=w_ps)

                    def emit_proj(s):
                        n0 = s * SLAB
                        last = s == NSLAB - 1
                        for t in range(NSUB):
                            o_ps = osp.tile([128, C], f32, tag="o", name="o_ps")
                            for p in range(4):
                                nc.tensor.matmul(
                                    o_ps,
                                    vt[:, p, n0 + t * 128 : n0 + (t + 1) * 128],
                                    wpp[:, p, :],
                                    start=(p == 0), stop=(p == 3),
                                )
                            o_sb = osb.tile([128, C], f32, tag="o_sb", name="o_sb")
                            if has_bp:
                                nc.vector.tensor_add(out=o_sb, in0=o_ps, in1=bp_t)
                                nc.sync.dma_start(
                                    out=out[:][n0 + t * 128 : n0 + (t + 1) * 128, :],
                                    in_=o_sb,
                                )
                            elif last:
                                # drain the tail in halves: DMA starts after half
                                # the copy, on alternating engines
                                for hh in range(2):
                                    csl = slice(hh * 256, (hh + 1) * 256)
                                    if (t + hh) % 2 == 0:
                                        nc.scalar.copy(out=o_sb[:, csl], in_=o_ps[:, csl])
                                    else:
                                        nc.vector.tensor_copy(
                                            out=o_sb[:, csl], in_=o_ps[:, csl]
                                        )
                                    nc.sync.dma_start(
                                        out=out[:][
                                            n0 + t * 128 : n0 + (t + 1) * 128, csl
                                        ],
                                        in_=o_sb[:, csl],
                                    )
                            else:
                                if t % 2 == 0:
                                    nc.scalar.copy(out=o_sb, in_=o_ps)
                                else:
                                    nc.vector.tensor_copy(out=o_sb, in_=o_ps)
                                nc.sync.dma_start(
                                    out=out[:][n0 + t * 128 : n0 + (t + 1) * 128, :],
                                    in_=o_sb,
                                )

                    for s in range(NSLAB):
                        emit_proj(s)

    return nc


_cache: dict = {}
last_results = None


def kernel(x, w_qkv, b_qkv, w_p, b_p, temperature):
    global last_results
    import os

    import ml_dtypes

    dt_name = os.environ.get("KSA_DT", "bf16")
    dt = {"bf16": bf16, "f32r": f32r}[dt_name]
    np_dt = ml_dtypes.bfloat16 if dt is bf16 else np.float32

    x = np.ascontiguousarray(np.asarray(x, dtype=np.float32))
    w_qkv = np.ascontiguousarray(np.asarray(w_qkv, dtype=np.float32).astype(np_dt))
    b_qkv = np.ascontiguousarray(np.asarray(b_qkv, dtype=np.float32))
    w_p = np.ascontiguousarray(np.asarray(w_p, dtype=np.float32).astype(np_dt))
    b_p = np.ascontiguousarray(np.asarray(b_p, dtype=np.float32))
    temperature = np.ascontiguousarray(np.asarray(temperature, dtype=np.float32))

    global _LDW_OPT
    _LDW_OPT = dt is f32r

    key = (bool(np.any(b_qkv)), bool(np.any(b_p)), dt_name)
    if key not in _cache:
        _cache[key] = _build(key[0], key[1], dt=dt)
    nc = _cache[key]

    in_maps = []
    for i in range(B):
        in_maps.append(
            {
                "xt": np.ascontiguousarray(x[i].T.astype(np_dt)),
                "w_qkv": w_qkv,
                "b_qkv": b_qkv,
                "w_p": w_p,
                "b_p": b_p,
                "temperature": temperature,
            }
        )

    trace = bool(int(os.environ.get("KSA_TRACE", "0")))
    res = run_bass_kernel_spmd(nc, in_maps, core_ids=list(range(B)), trace=trace)
    last_results = res
    return np.stack([res.results[i]["out"] for i in range(B)]).astype(np.float32)


# revision 48
# speedup vs baseline: 1.1449x; 1.0323x over previous
"""Channel self-attention (b=8, n=4096, c=512, h=8, d=64) on 8 trn2 cores.

Sharding: data-parallel over batch — core i computes batch element i.
Host pre-transposes each shard to x[b].T ([512, 4096], contiguous) so the
contraction dim (c) lands on SBUF partitions with no on-chip transposes,
and pre-casts x / w_qkv / w_p to the matmul dtype (bf16 or f32r; PSUM
accumulation is fp32 either way — bf16 end-to-end rel err ~9e-3 vs the
fp32 reference, tolerance 2e-2).

Key algebraic trick (phase 2): attention is folded into the output
projection.  out = y @ w_p with y[n,(h,d)] = sum_e attn_h[d,e] v[n,(h,e)]
equals v @ wp' where wp'[(h,e),j] = sum_d attn_h[d,e] w_p[(h,d),j].
wp' costs 4 pair-packed matmuls (2048 PE cycles) and removes the whole
attn@v pass (16K cycles), the 8 PE transposes of attn, and 32 large
PSUM->SBUF copies.

Per-core dataflow:
  phase 1, per 512-token slab:
     xT tile [128, 4cc, 512]  (DMA)
     q,k  [128tok, 512] = sum_cc xT[cc].T @ w_{q,k}[cc]      (xT stationary)
     vT   [128ch, 512tok] = sum_cc w_v[cc, echunk].T @ xT[cc] (w_v stationary)
     scores (PSUM-accumulated over all 32 subtiles, per head-pair):
        bf16: s_pair [128(2h x 64d), 128(2h x 64e)] += q[:,pair].T @ k[:,pair]
        f32r: 256-wide k window (f32r needs >=256 moving rows for full rate)
  softmax: extract 8 [64,64] head blocks (x temperature), batched softmax
     over the free dim, copy into block-diagonal attn tiles (2 heads/tile).
  phase 2:
     wp'[pair] [128(2h x 64e), 512] = blockdiag(attn)[pair].T @ w_p[pair]
     out [128tok, 512] = sum_pair vT[pair,tok].T @ wp'[pair] + b_p, DMA out.
"""

import numpy as np

import concourse.bass as bass
import concourse.mybir as mybir
import concourse.tile as tile
from concourse.bass_utils import run_bass_kernel_spmd
from concourse.vector_clock import ScopedClock

f32 = mybir.dt.float32
f32r = mybir.dt.float32r
bf16 = mybir.dt.bfloat16
AX = mybir.AxisListType
AF = mybir.ActivationFunctionType

B, N, C = 8, 4096, 512
H, D = 8, 64
CC = C // 128          # 4 contraction chunks
SLAB = 512             # tokens per phase loop iteration
NSLAB = N // SLAB      # 8
NSUB = SLAB // 128     # 4


# ---------------------------------------------------------------------------
# Workaround: this walrus build allows 1 sync wait per instruction (2 on
# EventSemaphore), but TileContext's tail attaches every end-of-kernel wait to
# a single Drain.  Redistribute onto single-wait EventSemaphore instructions.
def _drain_and_barrier_split(self, tick_clock, wait_clock):
    nc = self.nc
    dummy = mybir.InstDrain(name=f"I-waitprobe-{nc.next_id()}", ins=[], outs=[])
    dummy.engine = mybir.EngineType.SP
    wait_clock.add_sem_waits(dummy, ScopedClock({None: tick_clock.global_clock}))
    num2handle = {h.num: h for h in self.sems.allocated().values()}
    for w in dummy.sync_info.on_wait:
        assert w.wait_mode == "sem-ge-imm", w
        nc.sync.wait_ge(num2handle[w.id], w.wait_value)
    nc.sync.drain()
    nc.all_engine_barrier()
    assert self.sems is not None
    popped = nc._tile_sem_poison_stack.pop()
    assert popped is self._sem_poison
    nc.clear_and_free_semaphores(list(self.sems.allocated().values()))
    nc.all_engine_barrier()


tile.TileContext._drain_and_barrier = _drain_and_barrier_split


# Same walrus limit, applied generally: Tile's add_semaphores can attach
# several waits to one instruction.  Split the excess onto EventSemaphore
# instructions (capacity 2) inserted just before, on the same engine, at BIR
# JSON serialization time so both the compile and bass2jax paths see it.
def _split_excess_waits_json(j):
    import copy

    for fn in j.get("functions", []):
        for bb in fn.get("blocks", []):
            new_insts = []
            for ins in bb.get("instructions", []):
                si = ins.get("sync_info") or {}
                waits = si.get("on_wait") or []
                cap = 2 if ins.get("opcode") == "EventSemaphore" else 1
                if len(waits) > cap:
                    keep = waits[-cap:]
                    excess = waits[:-cap]
                    for i in range(0, len(excess), 2):
                        new_insts.append(
                            {
                                "engine": ins["engine"],
                                "ins": [],
                                "outs": [],
                                "name": f"{ins['name']}-wsp{i}",
                                "opcode": "EventSemaphore",
                                "sync_info": {
                                    "on_update": [],
                                    "on_wait": excess[i : i + 2],
                                },
                            }
                        )
                    si = copy.deepcopy(si)
                    si["on_wait"] = keep
                    ins["sync_info"] = si
                new_insts.append(ins)
            bb["instructions"] = new_insts
    return j


# Drop redundant PE weight loads: tile-legalize emits one InstLdweights per
# bf16 matmul, but adjacent q/k matmuls share the same stationary xt chunk —
# the second load re-streams identical weights (~53ns each on HW).  Safe to
# drop when the access pattern matches the previous Ldweights exactly, the
# instruction carries no sync, and only Matmults sit in between.
def _drop_redundant_ldweights_json(j):
    import json as _json

    for fn in j.get("functions", []):
        for bb in fn.get("blocks", []):
            insts = bb.get("instructions", [])
            keep = []
            prev_ldw_key = None
            for ins in insts:
                op = ins.get("opcode")
                eng = ins.get("engine")
                if eng == "PE" and op == "Ldweights":
                    si = ins.get("sync_info") or {}
                    key = _json.dumps(
                        [ins.get("ins"), ins.get("perf_mode"),
                         ins.get("is_transpose"), ins.get("tile_position")],
                        sort_keys=True,
                    )
                    if (
                        key == prev_ldw_key
                        and not si.get("on_wait")
                        and not si.get("on_update")
                    ):
                        continue  # redundant reload of identical weights
                    prev_ldw_key = key
                elif eng == "PE" and op == "Matmult":
                    pass  # matmuls don't disturb the loaded weights
                elif eng == "PE":
                    prev_ldw_key = None  # anything else on PE: be safe
                keep.append(ins)
            bb["instructions"] = keep
    return j


_orig_to_json_bytes = bass.Bass.to_json_bytes


def _patched_to_json_bytes(self):
    import json as _json
    import os as _os

    j = _json.loads(_orig_to_json_bytes(self))
    j = _split_excess_waits_json(j)
    if _os.environ.get("KSA_DEDUP_LDW", "1") == "1":
        j = _drop_redundant_ldweights_json(j)
    return _json.dumps(j).encode()


bass.Bass.to_json_bytes = _patched_to_json_bytes


# walrus's LDWEIGHTS optimization (pinned off in bass_utils) helps the f32r
# variant (~30us of per-matmul weight-load overhead, self-loading matmuls)
# but hard-errors on the explicit InstLdweights pairs bf16 matmuls lower to.
# Flip it per-build via the module-global _LDW_OPT at compile time.
import concourse.bass_utils as _BU

_LDW_OPT = False
_orig_run_command = _BU.run_command


def _run_command_ldwopt(argv, **kw):
    if _LDW_OPT:
        argv = [
            "--enable-ldw-opt=true" if x == "--enable-ldw-opt=false" else x
            for x in argv
        ]
    return _orig_run_command(argv, **kw)


_BU.run_command = _run_command_ldwopt
# ---------------------------------------------------------------------------


def _bcast_ap(handle, offset, ap):
    base = handle[:]
    return bass.AP(tensor=base.tensor, offset=offset, ap=ap)


def _build(
    has_bqkv: bool, has_bp: bool, repeat: int = 1, dt=bf16
) -> bass.Bass:
    nc = bass.Bass()

    xt = nc.dram_tensor("xt", [C, N], dt, kind="ExternalInput")
    w_qkv = nc.dram_tensor("w_qkv", [C, 3 * C], dt, kind="ExternalInput")
    b_qkv = nc.dram_tensor("b_qkv", [3 * C], f32, kind="ExternalInput")
    w_p = nc.dram_tensor("w_p", [C, C], dt, kind="ExternalInput")
    b_p = nc.dram_tensor("b_p", [C], f32, kind="ExternalInput")
    temp = nc.dram_tensor("temperature", [H, 1, 1], f32, kind="ExternalInput")
    if repeat > 1:
        # structurally distinguishes the repeat-variant HLO so the neuron
        # compile cache cannot alias it to the repeat=1 NEFF
        salt = nc.dram_tensor("salt", [repeat], f32, kind="ExternalInput")
    out = nc.dram_tensor("out", [N, C], f32, kind="ExternalOutput")

    # scores: bf16 runs 1 row/cycle at any moving size => exact 128-wide
    # 2-head windows; f32r needs >=256 moving rows => 256-wide windows.
    SN = 128 if dt is not f32r else 256

    xt_r = xt[:].rearrange("(cc p) n -> p cc n", p=128)
    wqkv_r = w_qkv[:].rearrange("(cc p) j -> p cc j", p=128)
    wp_r = w_p[:].rearrange("(cc p) j -> p cc j", p=128)

    with tile.TileContext(nc) as tc:
        with (
            tc.tile_pool(name="consts", bufs=1) as consts,
            tc.tile_pool(name="vtp", bufs=1) as vtp,
            tc.tile_pool(name="attnp", bufs=1) as attnp,
        ):
            # ---- constants ----
            # weights go on SWDGE (gpsimd) so they stream in parallel with the
            # HWDGE x-tile loads; q/k chunks split per-cc so PE starts early.
            # separate tiles per matrix so q matmuls gate only on wq's DMAs
            wq_t = consts.tile([128, CC, C], dt)
            wk_t = consts.tile([128, CC, C], dt)
            wv_t = consts.tile([128, CC, C], dt)
            # slab 0 of x rides HWDGE ahead of/with the first weight chunks
            # (the SWDGE path pays ~1.3us of gpsimd descriptor-gen per DMA,
            # which would gate the very first Ldweights); emission order =
            # first-subtile consumption order: xt[t0], wq0, wk0, xt[t1], ...
            # HWDGE descriptor processing is ~630ns/DMA and serial, so the
            # head wants few, medium-size DMAs in consumption order: x slab 0
            # and wq/wk in halves (2 cc chunks each), then slab 1, wv, wp
            xt0 = consts.tile([128, CC, SLAB], dt)
            xt1 = consts.tile([128, CC, SLAB], dt)
            if repeat > 1:
                # rep-parity ping-pong staging for slabs 0-1: consts-pool
                # tiles never collide with the per-rep SBUF stack churn, so
                # the next rep's x prefetch isn't WAR-blocked behind the
                # previous rep's phase-2 drain
                xt0b = consts.tile([128, CC, SLAB], dt)
                xt1b = consts.tile([128, CC, SLAB], dt)
            for hh in range(2):
                nc.sync.dma_start(
                    out=xt0[:, :, hh * 256 : (hh + 1) * 256],
                    in_=xt_r[:, :, hh * 256 : (hh + 1) * 256],
                )
                nc.sync.dma_start(
                    out=wq_t[:, 2 * hh : 2 * hh + 2, :],
                    in_=wqkv_r[:, 2 * hh : 2 * hh + 2, 0:C],
                )
                nc.sync.dma_start(
                    out=wk_t[:, 2 * hh : 2 * hh + 2, :],
                    in_=wqkv_r[:, 2 * hh : 2 * hh + 2, C : 2 * C],
                )
            nc.sync.dma_start(out=xt1, in_=xt_r[:, :, SLAB : 2 * SLAB])
            nc.sync.dma_start(out=wv_t, in_=wqkv_r[:, :, 2 * C : 3 * C])
            wp_t = consts.tile([128, CC, C], dt)
            nc.sync.dma_start(out=wp_t, in_=wp_r)
            temp_t = consts.tile([64, H], f32)
            nc.gpsimd.dma_start(out=temp_t, in_=_bcast_ap(temp, 0, [[0, 64], [1, H]]))
            if repeat > 1:
                salt_t = consts.tile([1, repeat], f32)
                nc.gpsimd.dma_start(out=salt_t, in_=salt[:][None, :])
            bd = consts.tile([128, 4, 128], dt)  # block-diag attn per pair
            zeros_t = consts.tile([128, 128], f32)
            nc.gpsimd.memset(zeros_t, 0.0)
            nc.vector.tensor_copy(
                out=bd,
                in_=bass.AP(
                    tensor=zeros_t.tensor,
                    offset=zeros_t.offset,
                    ap=[zeros_t.ap[0], [0, 4], zeros_t.ap[1]],
                ),
            )
            if has_bqkv:
                bqk_t = consts.tile([128, 2 * C], f32)
                nc.gpsimd.dma_start(
                    out=bqk_t, in_=_bcast_ap(b_qkv, 0, [[0, 128], [1, 2 * C]])
                )
                bv_t = consts.tile([128, CC], f32)
                nc.gpsimd.dma_start(
                    out=bv_t, in_=_bcast_ap(b_qkv, 2 * C, [[1, 128], [128, CC]])
                )
            if has_bp:
                bp_t = consts.tile([128, C], f32)
                nc.gpsimd.dma_start(
                    out=bp_t, in_=_bcast_ap(b_p, 0, [[0, 128], [1, C]])
                )

            for _rep in range(repeat):
                if _rep == 0:
                    xt_s0, xt_s1 = xt0, xt1
                else:
                    xt_s0, xt_s1 = (xt0, xt1) if _rep % 2 == 0 else (xt0b, xt1b)
                    # prefetch this rep's slabs 0-1 on SWDGE; the Pool queue
                    # reaches these right after the previous rep's slab DMAs,
                    # a full phase ahead of first use
                    for xt_sN, s in ((xt_s0, 0), (xt_s1, 1)):
                        for hh in range(2):
                            nc.gpsimd.dma_start(
                                out=xt_sN[:, :, hh * 256 : (hh + 1) * 256],
                                in_=xt_r[
                                    :, :, s * SLAB + hh * 256 : s * SLAB + (hh + 1) * 256
                                ],
                            )
                vt = vtp.tile([128, 4, N], dt)  # v^T: [pair-chunk rows, pair, token]

                with tc.tile_pool(name="spsum", bufs=1, space="PSUM") as spsum:
                    s_ps = [
                        spsum.tile([128, SN], f32, tag=f"s{p}", name=f"s{p}")
                        for p in range(4)
                    ]

                    # ================= phase 1 =================
                    with (
                        tc.tile_pool(name="xp", bufs=4) as xp,
                        tc.tile_pool(name="qkp", bufs=2) as qkp,
                        tc.tile_pool(name="qkps", bufs=1, space="PSUM") as qkps,
                        tc.tile_pool(name="vps", bufs=2, space="PSUM") as vps,
                    ):
                        NIT = NSLAB * NSUB

                        def emit_scores(q_sb, k_sb, it):
                            # stationary 2-head q block, moving k window
                            for p in range(4):
                                if SN == 128:
                                    kc0 = 128 * p
                                else:
                                    kc0 = 128 * p if p < 3 else 256
                                nc.tensor.matmul(
                                    s_ps[p],
                                    q_sb[:, p * 128 : (p + 1) * 128],
                                    k_sb[:, kc0 : kc0 + SN],
                                    start=(it == 0),
                                    stop=(it == NIT - 1),
                                )

                        def emit_v(s, xt_t):
                            # v^T chunks: w_v stationary, xT moving (N=512)
                            n0 = s * SLAB
                            for e in range(4):
                                v_ps = vps.tile([128, SLAB], f32, tag="v", name="v_ps")
                                for cc in range(CC):
                                    nc.tensor.matmul(
                                        v_ps,
                                        wv_t[:, cc, e * 128 : (e + 1) * 128],
                                        xt_t[:, cc, :],
                                        start=(cc == 0),
                                        stop=(cc == CC - 1),
                                    )
                                dst = vt[:, e, n0 : n0 + SLAB]
                                if has_bqkv:
                                    nc.vector.tensor_scalar_add(
                                        out=dst, in0=v_ps, scalar1=bv_t[:, e : e + 1]
                                    )
                                elif e % 2 == 0:
                                    nc.scalar.copy(out=dst, in_=v_ps)
                                else:
                                    nc.vector.tensor_copy(out=dst, in_=v_ps)

                        pending = None  # (q_sb, k_sb, it) one subtile behind
                        v_queue = []  # (s, xt_t), two slabs behind
                        for s in range(NSLAB):
                            n0 = s * SLAB
                            if s <= 1:
                                # staged via the consts-pool (parity) tiles
                                xt_t = xt_s0 if s == 0 else xt_s1
                            else:
                                xt_t = xp.tile([128, CC, SLAB], dt, name="xt_t")
                                # two halves: the next slab's first subtiles
                                # gate on half the transfer
                                hw_ = SLAB // 2
                                for hh in range(2):
                                    nc.gpsimd.dma_start(
                                        out=xt_t[:, :, hh * hw_ : (hh + 1) * hw_],
                                        in_=xt_r[
                                            :, :, n0 + hh * hw_ : n0 + (hh + 1) * hw_
                                        ],
                                    )

                            # q, k per 128-token subtile; scores lag one subtile so
                            # PE never waits on the q/k PSUM->SBUF copies.
                            for t in range(NSUB):
                                it = s * NSUB + t
                                q_ps = qkps.tile([128, C], f32, tag="q", name="q_ps")
                                k_ps = qkps.tile([128, C], f32, tag="k", name="k_ps")
                                for cc in range(CC):
                                    lhs = xt_t[:, cc, t * 128 : (t + 1) * 128]
                                    nc.tensor.matmul(
                                        q_ps, lhs, wq_t[:, cc, :],
                                        start=(cc == 0), stop=(cc == CC - 1),
                                    )
                                    nc.tensor.matmul(
                                        k_ps, lhs, wk_t[:, cc, :],
                                        start=(cc == 0), stop=(cc == CC - 1),
                                    )
                                q_sb = qkp.tile([128, C], dt, tag="q_sb", name="q_sb")
                                k_sb = qkp.tile([128, C], dt, tag="k_sb", name="k_sb")
                                if has_bqkv:
                                    nc.vector.tensor_add(out=q_sb, in0=q_ps, in1=bqk_t[:, 0:C])
                                    nc.vector.tensor_add(out=k_sb, in0=k_ps, in1=bqk_t[:, C : 2 * C])
                                else:
                                    nc.scalar.copy(out=q_sb, in_=q_ps)
                                    nc.vector.tensor_copy(out=k_sb, in_=k_ps)
                                if pending is not None:
                                    emit_scores(*pending)
                                pending = (q_sb, k_sb, it)

                            v_queue.append((s, xt_t))
                            if len(v_queue) > 2:
                                emit_v(*v_queue.pop(0))
                        emit_scores(*pending)
                        for args in v_queue:
                            emit_v(*args)

                    # ============ softmax (fused from PSUM) ============
                    # writes normalized bf16/f32r attn straight into the
                    # block-diagonal tiles bd[pair] (off-diag stays zero)
                    attn = attnp.tile([64, H, 64], f32)
                    m = attnp.tile([64, H], f32)
                    ssum = attnp.tile([64, H], f32)
                    for h in range(H):
                        p = h // 2
                        r0 = (h % 2) * 64
                        c0 = (h % 2) * 64 + (128 if (SN == 256 and p == 3) else 0)
                        blk = s_ps[p][r0 : r0 + 64, c0 : c0 + 64]
                        nc.vector.reduce_max(out=m[:, h : h + 1], in_=blk, axis=AX.X)
                        # m <- -(temp * max)
                        nc.vector.tensor_scalar(
                            out=m[:, h : h + 1], in0=m[:, h : h + 1],
                            scalar1=temp_t[:, h : h + 1], scalar2=-1.0,
                            op0=mybir.AluOpType.mult, op1=mybir.AluOpType.mult,
                        )
                        # attn_h = exp(temp*s - temp*max), row sums into ssum
                        nc.scalar.activation(
                            out=attn[:, h, :], in_=blk, func=AF.Exp,
                            bias=m[:, h : h + 1], scale=temp_t[:, h : h + 1],
                            accum_out=ssum[:, h : h + 1],
                        )
                    nc.vector.reciprocal(out=ssum, in_=ssum)
                    attn_b = attnp.tile([64, H, 64], dt)
                    for h in range(H):
                        # normalize then immediately stage into the block-diag
                        # tile, Act/DVE alternating, so bd pairs complete
                        # progressively and wp' matmuls can start early
                        nc.vector.tensor_scalar_mul(
                            out=attn_b[:, h, :],
                            in0=attn[:, h, :],
                            scalar1=ssum[:, h : h + 1],
                        )
                        o = (h % 2) * 64
                        if h % 2 == 0:
                            nc.scalar.copy(
                                out=bd[o : o + 64, h // 2, o : o + 64],
                                in_=attn_b[:, h, :],
                            )
                        else:
                            nc.vector.tensor_copy(
                                out=bd[o : o + 64, h // 2, o : o + 64],
                                in_=attn_b[:, h, :],
                            )

                # ================= phase 2 =================
                # wp'[pair] = blockdiag(attn)[pair].T @ w_p[pair], then
                # out[tok,:] = sum_pair vT[pair,tok].T @ wp'[pair]
                with (
                    tc.tile_pool(name="wpsb", bufs=1) as wpsb,
                    # osp opens before wps so it sits at the PSUM stack base:
                    # the next rep's first-touched pools (spsum/qkps) then
                    # collide with osp banks whose last chains retire several
                    # subtiles before the rep ends, not with the very tail
                    tc.tile_pool(name="osp", bufs=6, space="PSUM") as osp,
                    tc.tile_pool(name="wps", bufs=2, space="PSUM") as wps,
                    # osb bufs must stay even: copies alternate Act/DVE per
                    # subtile, so even reuse distance keeps WAW deps within
                    # one (in-order) engine instead of cross-engine ping-pong
                    tc.tile_pool(name="osb", bufs=4) as osb,
                ):
                    wpp = wpsb.tile([128, 4, C], dt)
                    for p in range(4):
                        w_ps = wps.tile([128, C], f32, tag="wpp", name="w_ps")
                        nc.tensor.matmul(
                            w_ps, bd[:, p, :], wp_t[:, p, :], start=True, stop=True
                        )
                        if p % 2 == 0:
                            nc.scalar.copy(out=wpp[:, p, :], in_=w_ps)
                        else:
                            nc.vector.tensor_copy(out=wpp[:, p, :], in_=w_ps)

                    def emit_proj(s):
                        n0 = s * SLAB
                        last = s == NSLAB - 1
                        for t in range(NSUB):
                            o_ps = osp.tile([128, C], f32, tag="o", name="o_ps")
                            for p in range(4):
                                nc.tensor.matmul(
                                    o_ps,
                                    vt[:, p, n0 + t * 128 : n0 + (t + 1) * 128],
                                    wpp[:, p, :],
                                    start=(p == 0), stop=(p == 3),
                                )
                            o_sb = osb.tile([128, C], f32, tag="o_sb", name="o_sb")
                            if has_bp:
                                nc.vector.tensor_add(out=o_sb, in0=o_ps, in1=bp_t)
                                nc.sync.dma_start(
                                    out=out[:][n0 + t * 128 : n0 + (t + 1) * 128, :],
                                    in_=o_sb,
                                )
                            elif last:
                                # drain the tail in halves: DMA starts after half
                                # the copy, on alternating engines
                                for hh in range(2):
                                    csl = slice(hh * 256, (hh + 1) * 256)
                                    if (t + hh) % 2 == 0:
                                        nc.scalar.copy(out=o_sb[:, csl], in_=o_ps[:, csl])
                                    else:
                                        nc.vector.tensor_copy(
                                            out=o_sb[:, csl], in_=o_ps[:, csl]
                                        )
                                    nc.sync.dma_start(
                                        out=out[:][
                                            n0 + t * 128 : n0 + (t + 1) * 128, csl
                                        ],
                                        in_=o_sb[:, csl],
                                    )
                            else:
                                if t % 2 == 0:
                                    nc.scalar.copy(out=o_sb, in_=o_ps)
                                else:
                                    nc.vector.tensor_copy(out=o_sb, in_=o_ps)
                                nc.sync.dma_start(
                                    out=out[:][n0 + t * 128 : n0 + (t + 1) * 128, :],
                                    in_=o_sb,
                                )

                    for s in range(NSLAB):
                        emit_proj(s)

    return nc


_cache: dict = {}
last_results = None


def kernel(x, w_qkv, b_qkv, w_p, b_p, temperature):
    global last_results
    import os

    import ml_dtypes

    dt_name = os.environ.get("KSA_DT", "bf16")
    dt = {"bf16": bf16, "f32r": f32r}[dt_name]
    np_dt = ml_dtypes.bfloat16 if dt is bf16 else np.float32

    x = np.ascontiguousarray(np.asarray(x, dtype=np.float32))
    w_qkv = np.ascontiguousarray(np.asarray(w_qkv, dtype=np.float32).astype(np_dt))
    b_qkv = np.ascontiguousarray(np.asarray(b_qkv, dtype=np.float32))
    w_p = np.ascontiguousarray(np.asarray(w_p, dtype=np.float32).astype(np_dt))
    b_p = np.ascontiguousarray(np.asarray(b_p, dtype=np.float32))
    temperature = np.ascontiguousarray(np.asarray(temperature, dtype=np.float32))

    global _LDW_OPT
    _LDW_OPT = dt is f32r

    key = (bool(np.any(b_qkv)), bool(np.any(b_p)), dt_name)
    if key not in _cache:
        _cache[key] = _build(key[0], key[1], dt=dt)
    nc = _cache[key]

    in_maps = []
    for i in range(B):
        in_maps.append(
            {
                "xt": np.ascontiguousarray(x[i].T.astype(np_dt)),
                "w_qkv": w_qkv,
                "b_qkv": b_qkv,
                "w_p": w_p,
                "b_p": b_p,
                "temperature": temperature,
            }
        )

    trace = bool(int(os.environ.get("KSA_TRACE", "0")))
    res = run_bass_kernel_spmd(nc, in_maps, core_ids=list(range(B)), trace=trace)
    last_results = res
    return np.stack([res.results[i]["out"] for i in range(B)]).astype(np.float32)


# revision 54
# speedup vs baseline: 1.1587x; 1.0121x over previous
"""Channel self-attention (b=8, n=4096, c=512, h=8, d=64) on 8 trn2 cores.

Sharding: data-parallel over batch — core i computes batch element i.
Host pre-transposes each shard to x[b].T ([512, 4096], contiguous) so the
contraction dim (c) lands on SBUF partitions with no on-chip transposes,
and pre-casts x / w_qkv / w_p to the matmul dtype (bf16 or f32r; PSUM
accumulation is fp32 either way — bf16 end-to-end rel err ~9e-3 vs the
fp32 reference, tolerance 2e-2).

Key algebraic trick (phase 2): attention is folded into the output
projection.  out = y @ w_p with y[n,(h,d)] = sum_e attn_h[d,e] v[n,(h,e)]
equals v @ wp' where wp'[(h,e),j] = sum_d attn_h[d,e] w_p[(h,d),j].
wp' costs 4 pair-packed matmuls (2048 PE cycles) and removes the whole
attn@v pass (16K cycles), the 8 PE transposes of attn, and 32 large
PSUM->SBUF copies.

Per-core dataflow:
  phase 1, per 512-token slab:
     xT tile [128, 4cc, 512]  (DMA)
     q,k  [128tok, 512] = sum_cc xT[cc].T @ w_{q,k}[cc]      (xT stationary)
     vT   [128ch, 512tok] = sum_cc w_v[cc, echunk].T @ xT[cc] (w_v stationary)
     scores (PSUM-accumulated over all 32 subtiles, per head-pair):
        bf16: s_pair [128(2h x 64d), 128(2h x 64e)] += q[:,pair].T @ k[:,pair]
        f32r: 256-wide k window (f32r needs >=256 moving rows for full rate)
  softmax: extract 8 [64,64] head blocks (x temperature), batched softmax
     over the free dim, copy into block-diagonal attn tiles (2 heads/tile).
  phase 2:
     wp'[pair] [128(2h x 64e), 512] = blockdiag(attn)[pair].T @ w_p[pair]
     out [128tok, 512] = sum_pair vT[pair,tok].T @ wp'[pair] + b_p, DMA out.
"""

import numpy as np

import concourse.bass as bass
import concourse.mybir as mybir
import concourse.tile as tile
from concourse.bass_utils import run_bass_kernel_spmd
from concourse.vector_clock import ScopedClock

f32 = mybir.dt.float32
f32r = mybir.dt.float32r
bf16 = mybir.dt.bfloat16
AX = mybir.AxisListType
AF = mybir.ActivationFunctionType

B, N, C = 8, 4096, 512
H, D = 8, 64
CC = C // 128          # 4 contraction chunks
SLAB = 512             # tokens per phase loop iteration
NSLAB = N // SLAB      # 8
NSUB = SLAB // 128     # 4


# ---------------------------------------------------------------------------
# Workaround: this walrus build allows 1 sync wait per instruction (2 on
# EventSemaphore), but TileContext's tail attaches every end-of-kernel wait to
# a single Drain.  Redistribute onto single-wait EventSemaphore instructions.
def _drain_and_barrier_split(self, tick_clock, wait_clock):
    nc = self.nc
    dummy = mybir.InstDrain(name=f"I-waitprobe-{nc.next_id()}", ins=[], outs=[])
    dummy.engine = mybir.EngineType.SP
    wait_clock.add_sem_waits(dummy, ScopedClock({None: tick_clock.global_clock}))
    num2handle = {h.num: h for h in self.sems.allocated().values()}
    for w in dummy.sync_info.on_wait:
        assert w.wait_mode == "sem-ge-imm", w
        nc.sync.wait_ge(num2handle[w.id], w.wait_value)
    nc.sync.drain()
    nc.all_engine_barrier()
    assert self.sems is not None
    popped = nc._tile_sem_poison_stack.pop()
    assert popped is self._sem_poison
    nc.clear_and_free_semaphores(list(self.sems.allocated().values()))
    nc.all_engine_barrier()


tile.TileContext._drain_and_barrier = _drain_and_barrier_split


# Same walrus limit, applied generally: Tile's add_semaphores can attach
# several waits to one instruction.  Split the excess onto EventSemaphore
# instructions (capacity 2) inserted just before, on the same engine, at BIR
# JSON serialization time so both the compile and bass2jax paths see it.
def _split_excess_waits_json(j):
    import copy

    for fn in j.get("functions", []):
        for bb in fn.get("blocks", []):
            new_insts = []
            for ins in bb.get("instructions", []):
                si = ins.get("sync_info") or {}
                waits = si.get("on_wait") or []
                cap = 2 if ins.get("opcode") == "EventSemaphore" else 1
                if len(waits) > cap:
                    keep = waits[-cap:]
                    excess = waits[:-cap]
                    for i in range(0, len(excess), 2):
                        new_insts.append(
                            {
                                "engine": ins["engine"],
                                "ins": [],
                                "outs": [],
                                "name": f"{ins['name']}-wsp{i}",
                                "opcode": "EventSemaphore",
                                "sync_info": {
                                    "on_update": [],
                                    "on_wait": excess[i : i + 2],
                                },
                            }
                        )
                    si = copy.deepcopy(si)
                    si["on_wait"] = keep
                    ins["sync_info"] = si
                new_insts.append(ins)
            bb["instructions"] = new_insts
    return j


# Drop redundant PE weight loads: tile-legalize emits one InstLdweights per
# bf16 matmul, but adjacent q/k matmuls share the same stationary xt chunk —
# the second load re-streams identical weights (~53ns each on HW).  Safe to
# drop when the access pattern matches the previous Ldweights exactly, the
# instruction carries no sync, and only Matmults sit in between.
def _drop_redundant_ldweights_json(j):
    import json as _json

    for fn in j.get("functions", []):
        for bb in fn.get("blocks", []):
            insts = bb.get("instructions", [])
            keep = []
            prev_ldw_key = None
            for ins in insts:
                op = ins.get("opcode")
                eng = ins.get("engine")
                if eng == "PE" and op == "Ldweights":
                    si = ins.get("sync_info") or {}
                    key = _json.dumps(
                        [ins.get("ins"), ins.get("perf_mode"),
                         ins.get("is_transpose"), ins.get("tile_position")],
                        sort_keys=True,
                    )
                    if (
                        key == prev_ldw_key
                        and not si.get("on_wait")
                        and not si.get("on_update")
                    ):
                        continue  # redundant reload of identical weights
                    prev_ldw_key = key
                elif eng == "PE" and op == "Matmult":
                    pass  # matmuls don't disturb the loaded weights
                elif eng == "PE":
                    prev_ldw_key = None  # anything else on PE: be safe
                keep.append(ins)
            bb["instructions"] = keep
    return j


_orig_to_json_bytes = bass.Bass.to_json_bytes


def _patched_to_json_bytes(self):
    import json as _json
    import os as _os

    j = _json.loads(_orig_to_json_bytes(self))
    j = _split_excess_waits_json(j)
    if _os.environ.get("KSA_DEDUP_LDW", "1") == "1":
        j = _drop_redundant_ldweights_json(j)
    return _json.dumps(j).encode()


bass.Bass.to_json_bytes = _patched_to_json_bytes


# walrus's LDWEIGHTS optimization (pinned off in bass_utils) helps the f32r
# variant (~30us of per-matmul weight-load overhead, self-loading matmuls)
# but hard-errors on the explicit InstLdweights pairs bf16 matmuls lower to.
# Flip it per-build via the module-global _LDW_OPT at compile time.
import concourse.bass_utils as _BU

_LDW_OPT = False
_orig_run_command = _BU.run_command


def _run_command_ldwopt(argv, **kw):
    if _LDW_OPT:
        argv = [
            "--enable-ldw-opt=true" if x == "--enable-ldw-opt=false" else x
            for x in argv
        ]
    return _orig_run_command(argv, **kw)


_BU.run_command = _run_command_ldwopt
# ---------------------------------------------------------------------------


def _bcast_ap(handle, offset, ap):
    base = handle[:]
    return bass.AP(tensor=base.tensor, offset=offset, ap=ap)


def _build(
    has_bqkv: bool, has_bp: bool, repeat: int = 1, dt=bf16
) -> bass.Bass:
    nc = bass.Bass()

    xt = nc.dram_tensor("xt", [C, N], dt, kind="ExternalInput")
    w_qkv = nc.dram_tensor("w_qkv", [C, 3 * C], dt, kind="ExternalInput")
    b_qkv = nc.dram_tensor("b_qkv", [3 * C], f32, kind="ExternalInput")
    w_p = nc.dram_tensor("w_p", [C, C], dt, kind="ExternalInput")
    b_p = nc.dram_tensor("b_p", [C], f32, kind="ExternalInput")
    temp = nc.dram_tensor("temperature", [H, 1, 1], f32, kind="ExternalInput")
    if repeat > 1:
        # structurally distinguishes the repeat-variant HLO so the neuron
        # compile cache cannot alias it to the repeat=1 NEFF
        salt = nc.dram_tensor("salt", [repeat], f32, kind="ExternalInput")
    out = nc.dram_tensor("out", [N, C], f32, kind="ExternalOutput")

    # scores: bf16 runs 1 row/cycle at any moving size => exact 128-wide
    # 2-head windows; f32r needs >=256 moving rows => 256-wide windows.
    SN = 128 if dt is not f32r else 256

    xt_r = xt[:].rearrange("(cc p) n -> p cc n", p=128)
    wqkv_r = w_qkv[:].rearrange("(cc p) j -> p cc j", p=128)
    wp_r = w_p[:].rearrange("(cc p) j -> p cc j", p=128)

    with tile.TileContext(nc) as tc:
        with (
            tc.tile_pool(name="consts", bufs=1) as consts,
            tc.tile_pool(name="vtp", bufs=1) as vtp,
            tc.tile_pool(name="attnp", bufs=1) as attnp,
        ):
            # ---- constants ----
            # weights go on SWDGE (gpsimd) so they stream in parallel with the
            # HWDGE x-tile loads; q/k chunks split per-cc so PE starts early.
            # separate tiles per matrix so q matmuls gate only on wq's DMAs
            wq_t = consts.tile([128, CC, C], dt)
            wk_t = consts.tile([128, CC, C], dt)
            wv_t = consts.tile([128, CC, C], dt)
            # slab 0 of x rides HWDGE ahead of/with the first weight chunks
            # (the SWDGE path pays ~1.3us of gpsimd descriptor-gen per DMA,
            # which would gate the very first Ldweights); emission order =
            # first-subtile consumption order: xt[t0], wq0, wk0, xt[t1], ...
            # HWDGE descriptor processing is ~630ns/DMA and serial, so the
            # head wants few, medium-size DMAs in consumption order: x slab 0
            # and wq/wk in halves (2 cc chunks each), then slab 1, wv, wp
            xt0 = consts.tile([128, CC, SLAB], dt)
            xt1 = consts.tile([128, CC, SLAB], dt)
            if repeat > 1:
                # rep-parity ping-pong staging for slabs 0-1: consts-pool
                # tiles never collide with the per-rep SBUF stack churn, so
                # the next rep's x prefetch isn't WAR-blocked behind the
                # previous rep's phase-2 drain
                xt0b = consts.tile([128, CC, SLAB], dt)
                xt1b = consts.tile([128, CC, SLAB], dt)
            for hh in range(2):
                nc.sync.dma_start(
                    out=xt0[:, :, hh * 256 : (hh + 1) * 256],
                    in_=xt_r[:, :, hh * 256 : (hh + 1) * 256],
                )
                nc.sync.dma_start(
                    out=wq_t[:, 2 * hh : 2 * hh + 2, :],
                    in_=wqkv_r[:, 2 * hh : 2 * hh + 2, 0:C],
                )
                nc.sync.dma_start(
                    out=wk_t[:, 2 * hh : 2 * hh + 2, :],
                    in_=wqkv_r[:, 2 * hh : 2 * hh + 2, C : 2 * C],
                )
            nc.sync.dma_start(out=xt1, in_=xt_r[:, :, SLAB : 2 * SLAB])
            nc.sync.dma_start(out=wv_t, in_=wqkv_r[:, :, 2 * C : 3 * C])
            wp_t = consts.tile([128, CC, C], dt)
            nc.sync.dma_start(out=wp_t, in_=wp_r)
            temp_t = consts.tile([64, H], f32)
            nc.gpsimd.dma_start(out=temp_t, in_=_bcast_ap(temp, 0, [[0, 64], [1, H]]))
            if repeat > 1:
                salt_t = consts.tile([1, repeat], f32)
                nc.gpsimd.dma_start(out=salt_t, in_=salt[:][None, :])
            bd = consts.tile([128, 4, 128], dt)  # block-diag attn per pair
            zeros_t = consts.tile([128, 128], f32)
            nc.gpsimd.memset(zeros_t, 0.0)
            nc.vector.tensor_copy(
                out=bd,
                in_=bass.AP(
                    tensor=zeros_t.tensor,
                    offset=zeros_t.offset,
                    ap=[zeros_t.ap[0], [0, 4], zeros_t.ap[1]],
                ),
            )
            if has_bqkv:
                bqk_t = consts.tile([128, 2 * C], f32)
                nc.gpsimd.dma_start(
                    out=bqk_t, in_=_bcast_ap(b_qkv, 0, [[0, 128], [1, 2 * C]])
                )
                bv_t = consts.tile([128, CC], f32)
                nc.gpsimd.dma_start(
                    out=bv_t, in_=_bcast_ap(b_qkv, 2 * C, [[1, 128], [128, CC]])
                )
            if has_bp:
                bp_t = consts.tile([128, C], f32)
                nc.gpsimd.dma_start(
                    out=bp_t, in_=_bcast_ap(b_p, 0, [[0, 128], [1, C]])
                )

            for _rep in range(repeat):
                if _rep == 0:
                    xt_s0, xt_s1 = xt0, xt1
                else:
                    xt_s0, xt_s1 = (xt0, xt1) if _rep % 2 == 0 else (xt0b, xt1b)
                    # prefetch this rep's slabs 0-1 on SWDGE; the Pool queue
                    # reaches these right after the previous rep's slab DMAs,
                    # a full phase ahead of first use
                    for xt_sN, s in ((xt_s0, 0), (xt_s1, 1)):
                        for hh in range(2):
                            nc.gpsimd.dma_start(
                                out=xt_sN[:, :, hh * 256 : (hh + 1) * 256],
                                in_=xt_r[
                                    :, :, s * SLAB + hh * 256 : s * SLAB + (hh + 1) * 256
                                ],
                            )
                vt = vtp.tile([128, 4, N], dt)  # v^T: [pair-chunk rows, pair, token]

                with tc.tile_pool(name="spsum", bufs=1, space="PSUM") as spsum:
                    s_ps = [
                        spsum.tile([128, SN], f32, tag=f"s{p}", name=f"s{p}")
                        for p in range(4)
                    ]

                    # ================= phase 1 =================
                    with (
                        tc.tile_pool(name="xp", bufs=4) as xp,
                        tc.tile_pool(name="qkp", bufs=2) as qkp,
                        tc.tile_pool(name="qkps", bufs=1, space="PSUM") as qkps,
                        tc.tile_pool(name="vps", bufs=2, space="PSUM") as vps,
                    ):
                        NIT = NSLAB * NSUB

                        def emit_scores(q_sb, k_sb, it):
                            # stationary 2-head q block, moving k window
                            for p in range(4):
                                if SN == 128:
                                    kc0 = 128 * p
                                else:
                                    kc0 = 128 * p if p < 3 else 256
                                nc.tensor.matmul(
                                    s_ps[p],
                                    q_sb[:, p * 128 : (p + 1) * 128],
                                    k_sb[:, kc0 : kc0 + SN],
                                    start=(it == 0),
                                    stop=(it == NIT - 1),
                                )

                        def emit_v(s, xt_t):
                            # v^T chunks: w_v stationary, xT moving (N=512)
                            n0 = s * SLAB
                            for e in range(4):
                                v_ps = vps.tile([128, SLAB], f32, tag="v", name="v_ps")
                                for cc in range(CC):
                                    nc.tensor.matmul(
                                        v_ps,
                                        wv_t[:, cc, e * 128 : (e + 1) * 128],
                                        xt_t[:, cc, :],
                                        start=(cc == 0),
                                        stop=(cc == CC - 1),
                                    )
                                dst = vt[:, e, n0 : n0 + SLAB]
                                if has_bqkv:
                                    nc.vector.tensor_scalar_add(
                                        out=dst, in0=v_ps, scalar1=bv_t[:, e : e + 1]
                                    )
                                elif e % 2 == 0:
                                    nc.scalar.copy(out=dst, in_=v_ps)
                                else:
                                    nc.vector.tensor_copy(out=dst, in_=v_ps)

                        pending = None  # (q_sb, k_sb, it) one subtile behind
                        v_queue = []  # (s, xt_t), two slabs behind
                        for s in range(NSLAB):
                            n0 = s * SLAB
                            if s <= 1:
                                # staged via the consts-pool (parity) tiles
                                xt_t = xt_s0 if s == 0 else xt_s1
                            else:
                                xt_t = xp.tile([128, CC, SLAB], dt, name="xt_t")
                                # two halves: the next slab's first subtiles
                                # gate on half the transfer
                                hw_ = SLAB // 2
                                for hh in range(2):
                                    nc.gpsimd.dma_start(
                                        out=xt_t[:, :, hh * hw_ : (hh + 1) * hw_],
                                        in_=xt_r[
                                            :, :, n0 + hh * hw_ : n0 + (hh + 1) * hw_
                                        ],
                                    )

                            # q, k per 128-token subtile; scores lag one subtile so
                            # PE never waits on the q/k PSUM->SBUF copies.
                            for t in range(NSUB):
                                it = s * NSUB + t
                                q_ps = qkps.tile([128, C], f32, tag="q", name="q_ps")
                                k_ps = qkps.tile([128, C], f32, tag="k", name="k_ps")
                                for cc in range(CC):
                                    lhs = xt_t[:, cc, t * 128 : (t + 1) * 128]
                                    nc.tensor.matmul(
                                        q_ps, lhs, wq_t[:, cc, :],
                                        start=(cc == 0), stop=(cc == CC - 1),
                                    )
                                    nc.tensor.matmul(
                                        k_ps, lhs, wk_t[:, cc, :],
                                        start=(cc == 0), stop=(cc == CC - 1),
                                    )
                                q_sb = qkp.tile([128, C], dt, tag="q_sb", name="q_sb")
                                k_sb = qkp.tile([128, C], dt, tag="k_sb", name="k_sb")
                                if has_bqkv:
                                    nc.vector.tensor_add(out=q_sb, in0=q_ps, in1=bqk_t[:, 0:C])
                                    nc.vector.tensor_add(out=k_sb, in0=k_ps, in1=bqk_t[:, C : 2 * C])
                                else:
                                    nc.scalar.copy(out=q_sb, in_=q_ps)
                                    nc.vector.tensor_copy(out=k_sb, in_=k_ps)
                                if pending is not None:
                                    emit_scores(*pending)
                                pending = (q_sb, k_sb, it)

                            v_queue.append((s, xt_t))
                            if len(v_queue) > 2:
                                emit_v(*v_queue.pop(0))
                        emit_scores(*pending)
                        for args in v_queue:
                            emit_v(*args)

                    # ============ softmax (fused from PSUM) ============
                    # writes normalized bf16/f32r attn straight into the
                    # block-diagonal tiles bd[pair] (off-diag stays zero)
                    attn = attnp.tile([64, H, 64], f32)
                    m = attnp.tile([64, H], f32)
                    ssum = attnp.tile([64, H], f32)
                    for h in range(H):
                        p = h // 2
                        r0 = (h % 2) * 64
                        c0 = (h % 2) * 64 + (128 if (SN == 256 and p == 3) else 0)
                        blk = s_ps[p][r0 : r0 + 64, c0 : c0 + 64]
                        nc.vector.reduce_max(out=m[:, h : h + 1], in_=blk, axis=AX.X)
                        # m <- -(temp * max)
                        nc.vector.tensor_scalar(
                            out=m[:, h : h + 1], in0=m[:, h : h + 1],
                            scalar1=temp_t[:, h : h + 1], scalar2=-1.0,
                            op0=mybir.AluOpType.mult, op1=mybir.AluOpType.mult,
                        )
                        # attn_h = exp(temp*s - temp*max), row sums into ssum
                        nc.scalar.activation(
                            out=attn[:, h, :], in_=blk, func=AF.Exp,
                            bias=m[:, h : h + 1], scale=temp_t[:, h : h + 1],
                            accum_out=ssum[:, h : h + 1],
                        )
                    nc.vector.reciprocal(out=ssum, in_=ssum)
                    attn_b = attnp.tile([64, H, 64], dt)
                    for h in range(H):
                        # normalize then immediately stage into the block-diag
                        # tile, Act/DVE alternating, so bd pairs complete
                        # progressively and wp' matmuls can start early
                        nc.vector.tensor_scalar_mul(
                            out=attn_b[:, h, :],
                            in0=attn[:, h, :],
                            scalar1=ssum[:, h : h + 1],
                        )
                        o = (h % 2) * 64
                        if h % 2 == 0:
                            nc.scalar.copy(
                                out=bd[o : o + 64, h // 2, o : o + 64],
                                in_=attn_b[:, h, :],
                            )
                        else:
                            nc.vector.tensor_copy(
                                out=bd[o : o + 64, h // 2, o : o + 64],
                                in_=attn_b[:, h, :],
                            )

                # ================= phase 2 =================
                # wp'[pair] = blockdiag(attn)[pair].T @ w_p[pair], then
                # out[tok,:] = sum_pair vT[pair,tok].T @ wp'[pair]
                with (
                    tc.tile_pool(name="wpsb", bufs=1) as wpsb,
                    # osp opens before wps so it sits at the PSUM stack base:
                    # the next rep's first-touched pools (spsum/qkps) then
                    # collide with osp banks whose last chains retire several
                    # subtiles before the rep ends, not with the very tail
                    tc.tile_pool(name="osp", bufs=6, space="PSUM") as osp,
                    tc.tile_pool(name="wps", bufs=2, space="PSUM") as wps,
                    # osb bufs must stay even: copies alternate Act/DVE per
                    # subtile, so even reuse distance keeps WAW deps within
                    # one (in-order) engine instead of cross-engine ping-pong
                    tc.tile_pool(name="osb", bufs=4) as osb,
                ):
                    wpp = wpsb.tile([128, 4, C], dt)
                    for p in range(4):
                        w_ps = wps.tile([128, C], f32, tag="wpp", name="w_ps")
                        nc.tensor.matmul(
                            w_ps, bd[:, p, :], wp_t[:, p, :], start=True, stop=True
                        )
                        if p % 2 == 0:
                            nc.scalar.copy(out=wpp[:, p, :], in_=w_ps)
                        else:
                            nc.vector.tensor_copy(out=wpp[:, p, :], in_=w_ps)

                    def emit_proj(s):
                        n0 = s * SLAB
                        last = s == NSLAB - 1
                        for t in range(NSUB):
                            o_ps = osp.tile([128, C], f32, tag="o", name="o_ps")
                            for p in range(4):
                                nc.tensor.matmul(
                                    o_ps,
                                    vt[:, p, n0 + t * 128 : n0 + (t + 1) * 128],
                                    wpp[:, p, :],
                                    start=(p == 0), stop=(p == 3),
                                )
                            o_sb = osb.tile([128, C], f32, tag="o_sb", name="o_sb")
                            if has_bp:
                                nc.vector.tensor_add(out=o_sb, in0=o_ps, in1=bp_t)
                                nc.sync.dma_start(
                                    out=out[:][n0 + t * 128 : n0 + (t + 1) * 128, :],
                                    in_=o_sb,
                                )
                            elif last:
                                # drain the tail in halves: DMA starts after half
                                # the copy, on alternating engines
                                for hh in range(2):
                                    csl = slice(hh * 256, (hh + 1) * 256)
                                    if (t + hh) % 2 == 0:
                                        nc.scalar.copy(out=o_sb[:, csl], in_=o_ps[:, csl])
                                    else:
                                        nc.vector.tensor_copy(
                                            out=o_sb[:, csl], in_=o_ps[:, csl]
                                        )
                                    nc.sync.dma_start(
                                        out=out[:][
                                            n0 + t * 128 : n0 + (t + 1) * 128, csl
                                        ],
                                        in_=o_sb[:, csl],
                                    )
                            else:
                                if t % 2 == 0:
                                    nc.scalar.copy(out=o_sb, in_=o_ps)
                                else:
                                    nc.vector.tensor_copy(out=o_sb, in_=o_ps)
                                nc.sync.dma_start(
                                    out=out[:][n0 + t * 128 : n0 + (t + 1) * 128, :],
                                    in_=o_sb,
                                )

                    for s in range(NSLAB):
                        emit_proj(s)

    return nc


_cache: dict = {}
last_results = None


def kernel(x, w_qkv, b_qkv, w_p, b_p, temperature):
    global last_results
    import os

    import ml_dtypes

    dt_name = os.environ.get("KSA_DT", "bf16")
    dt = {"bf16": bf16, "f32r": f32r}[dt_name]
    np_dt = ml_dtypes.bfloat16 if dt is bf16 else np.float32

    x = np.ascontiguousarray(np.asarray(x, dtype=np.float32))
    w_qkv = np.ascontiguousarray(np.asarray(w_qkv, dtype=np.float32).astype(np_dt))
    b_qkv = np.ascontiguousarray(np.asarray(b_qkv, dtype=np.float32))
    w_p = np.ascontiguousarray(np.asarray(w_p, dtype=np.float32).astype(np_dt))
    b_p = np.ascontiguousarray(np.asarray(b_p, dtype=np.float32))
    temperature = np.ascontiguousarray(np.asarray(temperature, dtype=np.float32))

    global _LDW_OPT
    _LDW_OPT = dt is f32r

    key = (bool(np.any(b_qkv)), bool(np.any(b_p)), dt_name)
    if key not in _cache:
        _cache[key] = _build(key[0], key[1], dt=dt)
    nc = _cache[key]

    in_maps = []
    for i in range(B):
        in_maps.append(
            {
                "xt": np.ascontiguousarray(x[i].T.astype(np_dt)),
                "w_qkv": w_qkv,
                "b_qkv": b_qkv,
                "w_p": w_p,
                "b_p": b_p,
                "temperature": temperature,
            }
        )

    trace = bool(int(os.environ.get("KSA_TRACE", "0")))
    res = run_bass_kernel_spmd(nc, in_maps, core_ids=list(range(B)), trace=trace)
    last_results = res
    return np.stack([res.results[i]["out"] for i in range(B)]).astype(np.float32)
